# revision 14
# baseline (speedup 1.0000x reference)
"""DRIM layer (distorted Rytov inverse-scattering iteration) on Trainium2.

Optimized single-core program replicated SPMD on 8 cores.  Key design:
  - all bulk matrix state (Z, factors, rhs, H) stored bf16 in SBUF/DRAM;
    fp32 PSUM accumulation everywhere (validated end-to-end ~1e-3)
  - elementwise work split across DVE (vector) and Pool (gpsimd) engines
  - complex products via plane-swapped (-im|re) rhs copies so each complex
    matmul is 2 wide PSUM-accumulating matmuls, one combine op
  - sin/cos range reduction via one fused (x+pi mod 2pi) tensor_scalar
  - activation-table churn avoided (two-pass Z build: sqrt pass, sin pass)
  - Newton block inversions emitted interleaved with trailing updates
  - pivot-row transposes via XBAR DMA-transpose loads (no PE transposes)
  - Gram accumulated over 4-row-chunk quads in PSUM
"""
import math
import numpy as np

import concourse.bass as bass
import concourse.bacc as bacc
import concourse.bass_isa as bass_isa
import concourse.mybir as mybir
import concourse.tile as tile
from concourse.bass_utils import run_bass_kernel_spmd

F32 = mybir.dt.float32
F32R = mybir.dt.float32r
BF16 = mybir.dt.bfloat16
U8 = mybir.dt.uint8
I32 = mybir.dt.int32
AF = mybir.ActivationFunctionType
ALU = mybir.AluOpType
AXX = mybir.AxisListType.X

M = 48
N = M * M
NB = N // 128               # 18
TX = RX = 40
L16 = 1600                  # 40x40 links incl. zero-weighted diagonal
LPAD = 1664
LB = LPAD // 128            # 13
RW = 256
DOI = 3.0
WL = 0.125
K0 = 2.0 * math.pi / WL
IMP = 120.0 * math.pi
GRID_LEN = DOI / M
GRID_RADIUS = math.sqrt(GRID_LEN ** 2 / math.pi)
NOISE = 1e-6

def _j1s(x):
    t2 = (x / 3.0) ** 2
    return x * (0.5 - 0.56249985*t2 + 0.21093573*t2**2 - 0.03954289*t2**3
                + 0.00443319*t2**4 - 0.00031761*t2**5 + 0.00001109*t2**6)

def _y1s(x):
    t2 = (x / 3.0) ** 2
    p = (-0.6366198 + 0.2212091*t2 + 2.1682709*t2**2 - 1.3164827*t2**3
         + 0.3123951*t2**4 - 0.0400976*t2**5 + 0.0027873*t2**6)
    return ((2.0/math.pi) * x * math.log(0.5*x) * _j1s(x) + p) / x

X0C = K0 * GRID_RADIUS
GRID_AREA = 4.0*math.pi*GRID_RADIUS/(2.0*K0) * _j1s(X0C)
C1 = -IMP * math.pi * GRID_RADIUS / 2.0
C2 = _j1s(X0C)
C3R, C3I = _j1s(X0C), _y1s(X0C)
C1C2 = C1 * C2
ZD_RE = C1 * C3R
ZD_IM_C = C1 * C3I
SA = GRID_AREA * K0 * K0
TWO_PI = 2.0 * math.pi
INV_2PI = 1.0 / TWO_PI
LOG10E20 = 20.0 * math.log10(math.e)
CADD = 10.0 * math.log10(WL * WL / (4.0 * math.pi * IMP) / 1e-3)
C20L = 20.0 / math.log(10.0)

F0C = [0.79788456, -0.00000077, -0.00552740, -0.00009512]
THC = [-0.78539816, -0.04166397, -0.00003954, 0.00262573]
F0CS = [c * (3.0 ** k) * C1C2 for k, c in enumerate(F0C)]
THCS = [c * (3.0 ** k) for k, c in enumerate(THC)]

NEWTON_Z = 17
NEWTON_SPD = 14


class Mux:
    """Alternate elementwise ops between DVE (vector) and Pool (gpsimd)."""
    def __init__(self, nc):
        self.nc = nc
        self.i = 0

    def eng(self):
        self.i += 1
        return self.nc.vector if (self.i & 1) else self.nc.gpsimd


def build_program(alpha):
    nc = bacc.Bacc("TRN2", target_bir_lowering=False, num_devices=8)
    din = {}
    def inp(name, shape, dtype=F32):
        din[name] = nc.dram_tensor(name, shape, dtype, kind="ExternalInput")
    inp("geomS", [4, N]); inp("geomR", [4, N]); inp("scat_t", [128, NB])
    inp("bpack", [N, RW]); inp("gscT", [N, 80]); inp("dfpack", [40, 80])
    inp("tp40", [40, 40]); inp("id128", [128, 128]); inp("idu8", [128, 128], U8)
    out_chi = nc.dram_tensor("out_chi", [2 * N], F32, kind="ExternalOutput")
    xdbg = nc.dram_tensor("xdbg", [N, RW], BF16, kind="ExternalOutput")
    tfdbg = nc.dram_tensor("tfdbg", [40, 80], F32, kind="ExternalOutput")
    scr = {}
    scr["utdram"] = nc.dram_tensor("utdram", [N, 2 * N], BF16, kind="Internal")
    scr["htdram"] = nc.dram_tensor("htdram", [2 * N, LPAD], BF16, kind="Internal")
    scr["sdram"] = nc.dram_tensor("sdram", [L16], F32, kind="Internal")
    scr["wrdram"] = nc.dram_tensor("wrdram", [L16], F32, kind="Internal")
    scr["widram"] = nc.dram_tensor("widram", [L16], F32, kind="Internal")
    scr["srowdram"] = nc.dram_tensor("srowdram", [LPAD], F32, kind="Internal")
    scr["yrowdram"] = nc.dram_tensor("yrowdram", [LPAD], F32, kind="Internal")

    with tile.TileContext(nc) as tc:
        _body(nc, tc, din, out_chi, xdbg, tfdbg, scr, alpha)
    nc.compile()
    return nc


def _newton_scale(nc, work, pmisc, m, tag):
    """a = 1/(max rowsum)^2 of |m| (m symmetric) -> [128,1] f32 AP."""
    cs = work.tile([128, 1], F32, tag=f"nwcs_{tag}")
    nc.vector.tensor_reduce(cs[:], m[:], axis=AXX, op=ALU.add)
    nc.gpsimd.partition_all_reduce(cs[:], cs[:], 128, bass_isa.ReduceOp.max)
    a = work.tile([128, 1], F32, tag=f"nwa_{tag}")
    nc.vector.tensor_tensor(out=a[:], in0=cs[:], in1=cs[:], op=ALU.mult)
    nc.vector.reciprocal(a[:], a[:])
    return a


def _newton_cplx_steps(nc, work, pmm, pmisc, Dap, consts, iters):
    """Generator of emission closures for one complex Newton inversion.

    Dap: [128,256] bf16 (re|im) block, symmetric; V is written back to Dap.
    """
    st = {}

    def prologue():
        m1 = work.tile([128, 128], F32, tag="nw_m1")
        m2 = work.tile([128, 128], F32, tag="nw_m2")
        nc.scalar.activation(m1[:], Dap[:, 0:128], AF.Abs)
        nc.scalar.activation(m2[:], Dap[:, 128:256], AF.Abs)
        nc.vector.tensor_tensor(out=m1[:], in0=m1[:], in1=m2[:], op=ALU.max)
        a = _newton_scale(nc, work, pmisc, m1, "c")
        X = work.tile([128, RW], BF16, tag="nw_X")
        XB = work.tile([128, RW], BF16, tag="nw_XB")
        nc.vector.tensor_scalar(out=X[:, 0:128], in0=Dap[:, 0:128],
                                scalar1=a[:], scalar2=None, op0=ALU.mult)
        nc.vector.tensor_scalar(out=X[:, 128:256], in0=Dap[:, 128:256],
                                scalar1=a[:], scalar2=-1.0, op0=ALU.mult,
                                op1=ALU.mult)
        nc.gpsimd.tensor_scalar(out=XB[:, 0:128], in0=Dap[:, 128:256],
                                scalar1=a[:], scalar2=None, op0=ALU.mult)
        nc.gpsimd.tensor_scalar(out=XB[:, 128:256], in0=Dap[:, 0:128],
                                scalar1=a[:], scalar2=None, op0=ALU.mult)
        st["X"], st["XB"] = X, XB

    yield prologue

    def one_iter():
        X, XB = st["X"], st["XB"]
        PP = pmm.tile([128, RW], F32, tag="nw_PP")
        nc.tensor.matmul(PP[:], Dap[:, 0:128], X[:], start=True, stop=False)
        nc.tensor.matmul(PP[:], Dap[:, 128:256], XB[:], start=False, stop=True)
        R = work.tile([128, RW], BF16, tag="nw_R")
        RB = work.tile([128, RW], BF16, tag="nw_RB")
        nc.vector.tensor_tensor(out=R[:], in0=consts["Ip"][:], in1=PP[:],
                                op=ALU.subtract)
        nc.gpsimd.tensor_scalar(out=RB[:, 0:128], in0=R[:, 128:256],
                                scalar1=-1.0, scalar2=None, op0=ALU.mult)
        nc.gpsimd.tensor_copy(RB[:, 128:256], R[:, 0:128])
        QQ = pmm.tile([128, RW], F32, tag="nw_QQ")
        nc.tensor.matmul(QQ[:], X[:, 0:128], R[:], start=True, stop=False)
        nc.tensor.matmul(QQ[:], X[:, 128:256], RB[:], start=False, stop=True)
        nc.vector.tensor_tensor(out=X[:], in0=X[:], in1=QQ[:], op=ALU.add)
        nc.gpsimd.tensor_scalar(out=XB[:, 0:128], in0=X[:, 128:256],
                                scalar1=-1.0, scalar2=None, op0=ALU.mult)
        nc.gpsimd.tensor_copy(XB[:, 128:256], X[:, 0:128])

    for _ in range(iters):
        yield one_iter

    def final():
        nc.vector.tensor_copy(Dap[:], st["X"][:])

    yield final


def _newton_real_steps(nc, work, pmm, pmisc, Dap, consts, iters):
    """Same for a real symmetric [128,128] bf16 block; V written to Dap."""
    st = {}

    def prologue():
        m1 = work.tile([128, 128], F32, tag="nw2_m1")
        nc.scalar.activation(m1[:], Dap[:], AF.Abs)
        a = _newton_scale(nc, work, pmisc, m1, "r")
        X = work.tile([128, 128], BF16, tag="nw2_X")
        nc.vector.tensor_scalar(out=X[:], in0=Dap[:], scalar1=a[:],
                                scalar2=None, op0=ALU.mult)
        st["X"] = X

    yield prologue

    def one_iter():
        X = st["X"]
        PP = pmm.tile([128, 128], F32, tag="nw2_PP")
        nc.tensor.matmul(PP[:], Dap[:], X[:], start=True, stop=True)
        R = work.tile([128, 128], BF16, tag="nw2_R")
        nc.vector.tensor_tensor(out=R[:], in0=consts["Ib"][:], in1=PP[:],
                                op=ALU.subtract)
        QQ = pmm.tile([128, 128], F32, tag="nw2_QQ")
        nc.tensor.matmul(QQ[:], X[:], R[:], start=True, stop=True)
        nc.vector.tensor_tensor(out=X[:], in0=X[:], in1=QQ[:], op=ALU.add)

    for _ in range(iters):
        yield one_iter

    def final():
        nc.vector.tensor_copy(Dap[:], st["X"][:])

    yield final


def _body(nc, tc, din, out_chi, xdbg, tfdbg, scr, alpha):
    import contextlib
    ctx = contextlib.ExitStack()
    mux = Mux(nc)

    consts_pool = ctx.enter_context(tc.tile_pool(name="consts", bufs=1))
    id_s = consts_pool.tile([128, 128], F32)
    nc.sync.dma_start(id_s[:], din["id128"][:])
    idu_s = consts_pool.tile([128, 128], U8)
    nc.sync.dma_start(idu_s[:], din["idu8"][:])
    idb_s = consts_pool.tile([128, 128], BF16)      # +I bf16
    nc.vector.tensor_copy(idb_s[:], id_s[:])
    nidb_s = consts_pool.tile([128, 128], BF16)     # -I bf16
    nc.gpsimd.tensor_scalar(out=nidb_s[:], in0=id_s[:], scalar1=-1.0,
                            scalar2=None, op0=ALU.mult)
    Ip_s = consts_pool.tile([128, RW], BF16)        # (I|0)
    nc.vector.memset(Ip_s[:], 0.0)
    nc.vector.tensor_copy(Ip_s[:, 0:128], id_s[:])
    Iq_s = consts_pool.tile([128, RW], BF16)        # (0|I)
    nc.gpsimd.memset(Iq_s[:], 0.0)
    nc.gpsimd.tensor_copy(Iq_s[:, 128:256], id_s[:])
    scat_s = consts_pool.tile([128, NB], F32)
    nc.sync.dma_start(scat_s[:], din["scat_t"][:])

    zdi_s = consts_pool.tile([128, NB], BF16)
    fsc_s = consts_pool.tile([128, NB], F32)
    t0 = consts_pool.tile([128, NB], F32)
    nc.vector.tensor_scalar(out=t0[:], in0=scat_s[:], scalar1=-1.0,
                            scalar2=None, op0=ALU.add)
    nc.vector.reciprocal(t0[:], t0[:])
    nc.vector.tensor_scalar(out=fsc_s[:], in0=t0[:], scalar1=(IMP / K0),
                            scalar2=None, op0=ALU.mult)
    nc.vector.tensor_tensor(out=t0[:], in0=t0[:], in1=scat_s[:], op=ALU.mult)
    nc.vector.tensor_scalar(out=zdi_s[:], in0=t0[:], scalar1=-(IMP / K0),
                            scalar2=ZD_IM_C, op0=ALU.mult, op1=ALU.add)
    zdr_c = consts_pool.tile([128, 1], BF16)
    nc.vector.memset(zdr_c[:], float(ZD_RE))
    npi_c = consts_pool.tile([128, 1], F32)
    nc.vector.memset(npi_c[:], -math.pi)
    pi2_c = consts_pool.tile([128, 1], F32)
    nc.vector.memset(pi2_c[:], math.pi / 2.0)

    bf_pool = ctx.enter_context(tc.tile_pool(name="bf", bufs=1))
    BF = [bf_pool.tile([128, RW], BF16, tag=f"bf{i}", name=f"bf{i}")
          for i in range(NB)]

    consts = {"Ip": Ip_s, "Iq": Iq_s, "Ib": idb_s, "nIb": nidb_s}

    with tc.tile_pool(name="tri", bufs=1) as tri:
        ZR = [tri.tile([128, (NB - i) * RW], BF16, tag=f"zr{i}", name=f"zr{i}")
              for i in range(NB)]

        # ---------------- P1: Z build ----------------
        with (
            tc.tile_pool(name="zb_geom", bufs=2) as gpool,
            tc.tile_pool(name="zb_scr", bufs=1) as spool,
            tc.tile_pool(name="zb_work", bufs=2) as work,
            tc.tile_pool(name="zb_psum", bufs=3, space="PSUM") as pz,
        ):
            th_s = spool.tile([128, N], F32, name="th_s")
            amp_s = spool.tile([128, N], BF16, name="amp_s")
            for k in range(NB):
                r0 = 128 * k
                Wr = (NB - k) * 128
                gS0 = gpool.tile([4, 128], F32, tag="gS0", name="gS0")
                nc.sync.dma_start(gS0[:], din["geomS"][:, r0:r0+128])
                gS = gpool.tile([4, 128], F32R, tag="gS", name="gS")
                nc.vector.tensor_copy(gS[:], gS0[:])
                # pass A: distances, polynomials, amplitude, phase
                for c in range(0, Wr, 512):
                    w = min(512, Wr - c)
                    gR0 = work.tile([4, 512], F32, tag="gR0")
                    nc.sync.dma_start(gR0[:, 0:w], din["geomR"][:, r0+c:r0+c+w])
                    gR = work.tile([4, 512], F32R, tag="gR")
                    nc.vector.tensor_copy(gR[:, 0:w], gR0[:, 0:w])
                    pd = pz.tile([128, 512], F32, tag="zb_pd")
                    nc.tensor.matmul(pd[:, 0:w], gS[:], gR[:, 0:w],
                                     start=True, stop=True)
                    dsq = work.tile([128, 512], F32, tag="zb_dsq")
                    nc.vector.tensor_scalar(out=dsq[:, 0:w], in0=pd[:, 0:w],
                                            scalar1=0.002, scalar2=None,
                                            op0=ALU.max)
                    x = work.tile([128, 512], F32, tag="zb_x")
                    nc.scalar.activation(x[:, 0:w], dsq[:, 0:w], AF.Sqrt,
                                         scale=float(K0 * K0))
                    sp = work.tile([128, 512], F32, tag="zb_sp")
                    nc.vector.reciprocal(sp[:, 0:w], x[:, 0:w])
                    s2 = work.tile([128, 512], F32, tag="zb_s2")
                    nc.gpsimd.tensor_tensor(out=s2[:, 0:w], in0=sp[:, 0:w],
                                            in1=sp[:, 0:w], op=ALU.mult)
                    t1 = work.tile([128, 512], F32, tag="zb_t1")
                    nc.vector.tensor_scalar(out=t1[:, 0:w], in0=sp[:, 0:w],
                                            scalar1=THCS[1], scalar2=THCS[0],
                                            op0=ALU.mult, op1=ALU.add)
                    t2 = work.tile([128, 512], F32, tag="zb_t2")
                    nc.gpsimd.tensor_scalar(out=t2[:, 0:w], in0=sp[:, 0:w],
                                            scalar1=THCS[3], scalar2=THCS[2],
                                            op0=ALU.mult, op1=ALU.add)
                    nc.vector.tensor_tensor(out=t1[:, 0:w], in0=t1[:, 0:w],
                                            in1=x[:, 0:w], op=ALU.add)
                    nc.gpsimd.tensor_tensor(out=t2[:, 0:w], in0=t2[:, 0:w],
                                            in1=s2[:, 0:w], op=ALU.mult)
                    nc.vector.tensor_tensor(out=th_s[:, c:c+w], in0=t1[:, 0:w],
                                            in1=t2[:, 0:w], op=ALU.add)
                    u1 = work.tile([128, 512], F32, tag="zb_u1")
                    nc.gpsimd.tensor_scalar(out=u1[:, 0:w], in0=sp[:, 0:w],
                                            scalar1=F0CS[1], scalar2=F0CS[0],
                                            op0=ALU.mult, op1=ALU.add)
                    u2 = work.tile([128, 512], F32, tag="zb_u2")
                    nc.vector.tensor_scalar(out=u2[:, 0:w], in0=sp[:, 0:w],
                                            scalar1=F0CS[3], scalar2=F0CS[2],
                                            op0=ALU.mult, op1=ALU.add)
                    sqx = work.tile([128, 512], F32, tag="zb_sqx")
                    nc.scalar.activation(sqx[:, 0:w], sp[:, 0:w], AF.Sqrt)
                    nc.vector.tensor_tensor(out=u2[:, 0:w], in0=u2[:, 0:w],
                                            in1=s2[:, 0:w], op=ALU.mult)
                    nc.gpsimd.tensor_tensor(out=u1[:, 0:w], in0=u1[:, 0:w],
                                            in1=u2[:, 0:w], op=ALU.add)
                    nc.gpsimd.tensor_tensor(out=amp_s[:, c:c+w],
                                            in0=u1[:, 0:w], in1=sqx[:, 0:w],
                                            op=ALU.mult)
                # pass B: sines into ZR row (strided per-plane writes)
                for c in range(0, Wr, 512):
                    w = min(512, Wr - c)
                    nblk = w // 128
                    u = work.tile([128, 512], F32, tag="zb_u")
                    nc.vector.tensor_scalar(out=u[:, 0:w], in0=th_s[:, c:c+w],
                                            scalar1=INV_2PI, scalar2=None,
                                            op0=ALU.mult)
                    ki = work.tile([128, 512], I32, tag="zb_ki")
                    nc.vector.tensor_copy(ki[:, 0:w], u[:, 0:w])
                    mf = work.tile([128, 512], F32, tag="zb_mf")
                    nc.vector.tensor_copy(mf[:, 0:w], ki[:, 0:w])
                    sa = work.tile([128, 512], F32, tag="zb_sa")
                    nc.vector.scalar_tensor_tensor(
                        out=sa[:, 0:w], in0=mf[:, 0:w], scalar=-TWO_PI,
                        in1=th_s[:, c:c+w], op0=ALU.mult, op1=ALU.add)
                    sinr = work.tile([128, 512], F32, tag="zb_sin")
                    nc.scalar.activation(sinr[:, 0:w], sa[:, 0:w], AF.Sin)
                    u2 = work.tile([128, 512], F32, tag="zb_u2")
                    nc.gpsimd.tensor_scalar(out=u2[:, 0:w], in0=u[:, 0:w],
                                            scalar1=0.25, scalar2=None,
                                            op0=ALU.add)
                    ki2 = work.tile([128, 512], I32, tag="zb_ki2")
                    nc.vector.tensor_copy(ki2[:, 0:w], u2[:, 0:w])
                    mf2 = work.tile([128, 512], F32, tag="zb_mf2")
                    nc.vector.tensor_copy(mf2[:, 0:w], ki2[:, 0:w])
                    sa2 = work.tile([128, 512], F32, tag="zb_sa2")
                    nc.vector.scalar_tensor_tensor(
                        out=sa2[:, 0:w], in0=mf2[:, 0:w], scalar=-TWO_PI,
                        in1=th_s[:, c:c+w], op0=ALU.mult, op1=ALU.add)
                    cosr = work.tile([128, 512], F32, tag="zb_cos")
                    nc.scalar.activation(cosr[:, 0:w], sa2[:, 0:w], AF.Sin,
                                         bias=pi2_c[:])
                    zr3 = ZR[k][:, 2*c:2*c+nblk*RW].rearrange(
                        "p (n t) -> p n t", t=RW)
                    s3 = sinr[:, 0:w].rearrange("p (n t) -> p n t", t=128)
                    c3 = cosr[:, 0:w].rearrange("p (n t) -> p n t", t=128)
                    a3 = amp_s[:, c:c+w].rearrange("p (n t) -> p n t", t=128)
                    nc.gpsimd.tensor_tensor(out=zr3[:, :, 0:128], in0=c3,
                                            in1=a3, op=ALU.mult)
                    nc.vector.tensor_tensor(out=zr3[:, :, 128:256], in0=s3,
                                            in1=a3, op=ALU.mult)
                # diagonal overrides
                nc.vector.copy_predicated(
                    ZR[k][:, 0:128], idu_s[:],
                    zdr_c[:].broadcast_to([128, 128]))
                nc.vector.copy_predicated(
                    ZR[k][:, 128:256], idu_s[:],
                    zdi_s[:, k:k+1].broadcast_to([128, 128]))

        # ---------------- P2: block LDL^T ----------------
        with (
            tc.tile_pool(name="lu_zb", bufs=1) as zbpool,
            tc.tile_pool(name="lu_work", bufs=2) as work,
            tc.tile_pool(name="lu_nw", bufs=1) as nwork,
            tc.tile_pool(name="lu_pmm", bufs=1, space="PSUM") as pmm,
            tc.tile_pool(name="lu_pup", bufs=3, space="PSUM") as pup,
        ):
            pmisc = None
            ZB = zbpool.tile([128, (NB - 1) * RW], BF16, name="zbswap")
            BFB = zbpool.tile([128, RW], BF16, name="bfbswap")
            ldtmp = work.tile([128, RW], F32, tag="ldtmp")
            for i in range(NB):
                nc.sync.dma_start(ldtmp[:], din["bpack"][128*i:128*(i+1), :])
                nc.vector.tensor_copy(BF[i][:], ldtmp[:])
                ldtmp = work.tile([128, RW], F32, tag="ldtmp")

            pending = []

            def drain(n):
                for _ in range(min(n, len(pending))):
                    pending.pop(0)()

            for step in _newton_cplx_steps(nc, nwork, pmm, pmisc,
                                           ZR[0][:, 0:RW], consts, NEWTON_Z):
                step()

            for k in range(NB):
                nr = NB - 1 - k      # trailing rows
                if nr > 0:
                    # swapped pivot row (-im|re) for blocks k+1..17
                    zb3 = ZB[:, 0:nr*RW].rearrange("p (n t) -> p n t", t=RW)
                    zr3 = ZR[k][:, RW:(nr+1)*RW].rearrange(
                        "p (n t) -> p n t", t=RW)
                    nc.vector.tensor_scalar(out=zb3[:, :, 0:128],
                                            in0=zr3[:, :, 128:256],
                                            scalar1=-1.0, scalar2=None,
                                            op0=ALU.mult)
                    nc.gpsimd.tensor_copy(zb3[:, :, 128:256], zr3[:, :, 0:128])
                    # swapped pivot rhs
                    nc.vector.tensor_scalar(out=BFB[:, 0:128],
                                            in0=BF[k][:, 128:256],
                                            scalar1=-1.0, scalar2=None,
                                            op0=ALU.mult)
                    nc.gpsimd.tensor_copy(BFB[:, 128:256], BF[k][:, 0:128])
                    # store pivot row for backsolve (transposed on load)
                    nc.sync.dma_start(
                        scr["utdram"][128*k:128*(k+1), RW*(k+1):RW*NB],
                        ZR[k][:, RW:(nr+1)*RW])
                for i in range(k + 1, NB):
                    off = (i - k) * RW
                    zoff = (i - k - 1) * RW
                    PL = pmm.tile([128, RW], F32, tag="lu_PL")
                    nc.tensor.matmul(PL[:], ZR[k][:, 0:128],
                                     ZR[k][:, off:off+RW], start=True,
                                     stop=False)
                    nc.tensor.matmul(PL[:], ZR[k][:, 128:256],
                                     ZB[:, zoff:zoff+RW], start=False,
                                     stop=True)
                    LT = work.tile([128, RW], BF16, tag="lu_LT")
                    nc.scalar.copy(LT[:], PL[:])
                    # rhs update
                    PBf = pmm.tile([128, RW], F32, tag="lu_PBf")
                    nc.tensor.matmul(PBf[:], LT[:, 0:128], BF[k][:],
                                     start=True, stop=False)
                    nc.tensor.matmul(PBf[:], LT[:, 128:256], BFB[:],
                                     start=False, stop=True)
                    nc.vector.tensor_tensor(out=BF[i][:], in0=BF[i][:],
                                            in1=PBf[:], op=ALU.subtract)
                    # trailing row update, 512-wide chunks
                    Wi = (NB - i) * RW
                    for c in range(0, Wi, 512):
                        w = min(512, Wi - c)
                        PU = pup.tile([128, 512], F32, tag="lu_PU")
                        nc.tensor.matmul(PU[:, 0:w], LT[:, 0:128],
                                         ZR[k][:, off+c:off+c+w],
                                         start=True, stop=False)
                        nc.tensor.matmul(PU[:, 0:w], LT[:, 128:256],
                                         ZB[:, zoff+c:zoff+c+w],
                                         start=False, stop=True)
                        mux.i += 1
                        if mux.i & 1:
                            nc.vector.tensor_tensor(out=ZR[i][:, c:c+w],
                                                    in0=ZR[i][:, c:c+w],
                                                    in1=PU[:, 0:w],
                                                    op=ALU.subtract)
                        else:
                            aptmp = work.tile([128, 512], BF16, tag="ap_tmp")
                            nc.scalar.copy(aptmp[:, 0:w], PU[:, 0:w])
                            nc.gpsimd.tensor_tensor(out=ZR[i][:, c:c+w],
                                                    in0=ZR[i][:, c:c+w],
                                                    in1=aptmp[:, 0:w],
                                                    op=ALU.subtract)
                    if i == k + 1:
                        pending = list(_newton_cplx_steps(
                            nc, nwork, pmm, pmisc, ZR[i][:, 0:RW], consts,
                            NEWTON_Z))
                        drain(2)
                    else:
                        drain(2)
                drain(len(pending))

        # ---------------- P3: back-substitution ----------------
        with (
            tc.tile_pool(name="bs_work", bufs=3) as work,
            tc.tile_pool(name="bs_pacc", bufs=2, space="PSUM") as pacc,
            tc.tile_pool(name="bs_pmm", bufs=2, space="PSUM") as pmm,
        ):
            for k in range(NB - 1, -1, -1):
                if k < NB - 1:
                    P1a = pacc.tile([128, RW], F32, tag="bs_p1")
                    P2a = pacc.tile([128, RW], F32, tag="bs_p2")
                    nc.tensor.matmul(P1a[:], consts["nIb"][:], BF[k][:],
                                     start=True, stop=False)
                    for j in range(k + 1, NB):
                        utr = work.tile([128, 128], BF16, tag="bs_utr")
                        uti = work.tile([128, 128], BF16, tag="bs_uti")
                        nc.sync.dma_start_transpose(
                            utr[:], scr["utdram"][128*k:128*(k+1),
                                                  RW*j:RW*j+128])
                        nc.sync.dma_start_transpose(
                            uti[:], scr["utdram"][128*k:128*(k+1),
                                                  RW*j+128:RW*j+256])
                        last = (j == NB - 1)
                        nc.tensor.matmul(P1a[:], utr[:], BF[j][:],
                                         start=False, stop=last)
                        nc.tensor.matmul(P2a[:], uti[:], BF[j][:],
                                         start=(j == k + 1), stop=last)
                    P2s = work.tile([128, RW], BF16, tag="bs_p2s")
                    nc.scalar.copy(P2s[:], P2a[:])
                    W = work.tile([128, RW], BF16, tag="bs_W")
                    nc.vector.tensor_tensor(out=W[:, 0:128],
                                            in0=P2s[:, 128:256],
                                            in1=P1a[:, 0:128],
                                            op=ALU.subtract)
                    nc.vector.scalar_tensor_tensor(
                        out=W[:, 128:256], in0=P1a[:, 128:256], scalar=-1.0,
                        in1=P2s[:, 0:128], op0=ALU.mult, op1=ALU.subtract)
                else:
                    W = BF[k]
                WB = work.tile([128, RW], BF16, tag="bs_WB")
                nc.vector.tensor_scalar(out=WB[:, 0:128], in0=W[:, 128:256],
                                        scalar1=-1.0, scalar2=None,
                                        op0=ALU.mult)
                nc.gpsimd.tensor_copy(WB[:, 128:256], W[:, 0:128])
                PS = pmm.tile([128, RW], F32, tag="bs_PS")
                nc.tensor.matmul(PS[:], ZR[k][:, 0:128], W[:],
                                 start=True, stop=False)
                nc.tensor.matmul(PS[:], ZR[k][:, 128:256], WB[:],
                                 start=False, stop=True)
                nc.scalar.copy(BF[k][:], PS[:])
                nc.sync.dma_start(xdbg[128*k:128*(k+1), :], BF[k][:])

    # ---------------- P4: total field, power model, weights ----------------
    late = ctx.enter_context(tc.tile_pool(name="late", bufs=1))
    dvec = late.tile([128, LB], F32)
    drep = late.tile([128, L16], F32)
    wrep_r = late.tile([128, L16], F32)
    wrep_i = late.tile([128, L16], F32)
    vsum = late.tile([128, 2 * NB], F32)
    lam = late.tile([128, 1], F32)
    st_ = late.tile([128, LB], F32)
    srep = late.tile([128, LPAD], F32)
    yrep = late.tile([128, LPAD], F32)
    with (
        tc.tile_pool(name="p4_work", bufs=2) as work,
        tc.tile_pool(name="p4_pacc", bufs=1, space="PSUM") as pacc,
        tc.tile_pool(name="p4_pmisc", bufs=1, space="PSUM") as pmisc,
    ):
        Ptf1 = pacc.tile([40, RW], F32, tag="tf_p1")
        Ptf2 = pacc.tile([40, RW], F32, tag="tf_p2")
        for i in range(NB):
            gt = work.tile([128, 80], F32, tag="tf_g")
            nc.sync.dma_start(gt[:], din["gscT"][128*i:128*(i+1), :])
            gtb = work.tile([128, 80], BF16, tag="tf_gb")
            mux.eng().tensor_copy(gtb[:], gt[:])
            stt = (i == 0); spp = (i == NB - 1)
            nc.tensor.matmul(Ptf1[:], gtb[:, 0:40], BF[i][:],
                             start=stt, stop=spp)
            nc.tensor.matmul(Ptf2[:], gtb[:, 40:80], BF[i][:],
                             start=stt, stop=spp)
        df = work.tile([40, 80], F32, tag="tf_df")
        nc.sync.dma_start(df[:], din["dfpack"][:])
        tfr = work.tile([40, 40], F32, tag="tfr")
        tfi = work.tile([40, 40], F32, tag="tfi")
        nc.vector.tensor_tensor(out=tfr[:], in0=df[:, 0:40],
                                in1=Ptf1[:, 0:40], op=ALU.add)
        nc.vector.tensor_tensor(out=tfr[:], in0=tfr[:],
                                in1=Ptf2[:, 128:168], op=ALU.subtract)
        nc.vector.tensor_tensor(out=tfi[:], in0=df[:, 40:80],
                                in1=Ptf1[:, 128:168], op=ALU.add)
        nc.vector.tensor_tensor(out=tfi[:], in0=tfi[:],
                                in1=Ptf2[:, 0:40], op=ALU.add)
        tfd = work.tile([40, 80], F32, tag="tf_out")
        nc.vector.tensor_copy(tfd[:, 0:40], tfr[:])
        nc.vector.tensor_copy(tfd[:, 40:80], tfi[:])
        nc.sync.dma_start(tfdbg[:], tfd[:])

        mask40 = work.tile([40, 40], F32, tag="mask40")
        zero40 = work.tile([40, 1], F32, tag="zero40")
        nc.vector.memset(mask40[:], 1.0)
        nc.vector.memset(zero40[:], 0.0)
        nc.vector.copy_predicated(mask40[:], idu_s[0:40, 0:40],
                                  zero40[:].broadcast_to([40, 40]))

        pw = work.tile([40, 40], F32, tag="pw")
        nc.vector.tensor_tensor(out=pw[:], in0=tfr[:], in1=tfr[:], op=ALU.mult)
        t1 = work.tile([40, 40], F32, tag="pw_t")
        nc.gpsimd.tensor_tensor(out=t1[:], in0=tfi[:], in1=tfi[:], op=ALU.mult)
        nc.vector.tensor_tensor(out=pw[:], in0=pw[:], in1=t1[:], op=ALU.add)
        amp = work.tile([40, 40], F32, tag="amp")
        nc.scalar.activation(amp[:], pw[:], AF.Sqrt)
        nc.vector.tensor_scalar(out=amp[:], in0=amp[:], scalar1=NOISE,
                                scalar2=None, op0=ALU.add)
        nc.scalar.activation(amp[:], amp[:], AF.Ln)
        tpi = work.tile([40, 40], F32, tag="tpi")
        nc.vector.tensor_scalar(out=tpi[:], in0=amp[:], scalar1=C20L,
                                scalar2=CADD, op0=ALU.mult, op1=ALU.add)
        rec = work.tile([40, 40], F32, tag="rec")
        nc.vector.reciprocal(rec[:], pw[:])
        wr = work.tile([40, 40], F32, tag="wr")
        nc.vector.scalar_tensor_tensor(out=wr[:], in0=tfr[:], scalar=SA,
                                       in1=rec[:], op0=ALU.mult, op1=ALU.mult)
        nc.vector.tensor_tensor(out=wr[:], in0=wr[:], in1=mask40[:],
                                op=ALU.mult)
        wi = work.tile([40, 40], F32, tag="wi")
        nc.vector.scalar_tensor_tensor(out=wi[:], in0=tfi[:], scalar=-SA,
                                       in1=rec[:], op0=ALU.mult, op1=ALU.mult)
        nc.vector.tensor_tensor(out=wi[:], in0=wi[:], in1=mask40[:],
                                op=ALU.mult)
        tp40 = work.tile([40, 40], F32, tag="tp40")
        nc.sync.dma_start(tp40[:], din["tp40"][:])
        d40 = work.tile([40, 40], F32, tag="d40")
        nc.vector.tensor_tensor(out=d40[:], in0=tp40[:], in1=tpi[:],
                                op=ALU.subtract)
        nc.vector.scalar_tensor_tensor(out=d40[:], in0=d40[:],
                                       scalar=1.0 / LOG10E20, in1=mask40[:],
                                       op0=ALU.mult, op1=ALU.mult)

        def t40_store(src, dram, name):
            pt = pmisc.tile([40, 40], F32, tag=f"t40p_{name}")
            nc.tensor.matmul(pt[:], src[:], id_s[0:40, 0:40], start=True,
                             stop=True)
            d = work.tile([40, 40], F32, tag=f"t40_{name}")
            nc.vector.tensor_copy(d[:], pt[:])
            nc.sync.dma_start(bass.AP(dram, 0, [[1, L16]]), d[:])

        t40_store(d40, scr["sdram"], "d")
        t40_store(wr, scr["wrdram"], "wr")
        t40_store(wi, scr["widram"], "wi")

        row = work.tile([1, L16], F32, tag="rowld")
        nc.sync.dma_start(row[:], bass.AP(scr["sdram"], 0, [[1, L16]]))
        nc.gpsimd.partition_broadcast(drep[:], row[:])
        row = work.tile([1, L16], F32, tag="rowld")
        nc.sync.dma_start(row[:], bass.AP(scr["wrdram"], 0, [[1, L16]]))
        nc.gpsimd.partition_broadcast(wrep_r[:], row[:])
        row = work.tile([1, L16], F32, tag="rowld")
        nc.sync.dma_start(row[:], bass.AP(scr["widram"], 0, [[1, L16]]))
        nc.gpsimd.partition_broadcast(wrep_i[:], row[:])

        nc.vector.memset(dvec[:], 0.0)
        nc.sync.dma_start(dvec[:, 0:12],
                          bass.AP(scr["sdram"], 0, [[1, 128], [128, 12]]))
        nc.sync.dma_start(dvec[0:64, 12:13],
                          bass.AP(scr["sdram"], 1536, [[1, 64]]))

    # ---------------- P5+P7: H build fused with Gram quads ----------------
    ga_pool = ctx.enter_context(tc.tile_pool(name="ga", bufs=1))
    GA = [ga_pool.tile([128, LPAD], F32, tag=f"ga{l}", name=f"ga{l}")
          for l in range(LB)]
    with (
        tc.tile_pool(name="p5_hq", bufs=1) as hqpool,
        tc.tile_pool(name="p5_work", bufs=1) as work,
        tc.tile_pool(name="p5_pg", bufs=4, space="PSUM") as pg,
    ):
        HQ = [hqpool.tile([128, LPAD], BF16, tag=f"hq{s}", name=f"hq{s}")
              for s in range(8)]
        for s in range(8):
            nc.gpsimd.memset(HQ[s][:, L16:LPAD], 0.0)
        nc.vector.memset(vsum[:], 0.0)

        def gram_round(q, nm):
            for l in range(LB):
                c0 = 128 * l
                for cc in range(c0, LPAD, 512):
                    cw = min(512, LPAD - cc)
                    pgt = pg.tile([128, 512], F32, tag="g_pg")
                    for m in range(nm):
                        nc.tensor.matmul(pgt[:, 0:cw], HQ[m][:, c0:c0+128],
                                         HQ[m][:, cc:cc+cw],
                                         start=(m == 0), stop=(m == nm - 1))
                    if q == 0:
                        nc.scalar.copy(GA[l][:, cc:cc+cw], pgt[:, 0:cw])
                    else:
                        nc.vector.tensor_tensor(out=GA[l][:, cc:cc+cw],
                                                in0=GA[l][:, cc:cc+cw],
                                                in1=pgt[:, 0:cw], op=ALU.add)

        for i in range(NB):
            sre = HQ[2 * (i % 4)]
            sim = HQ[2 * (i % 4) + 1]
            Gq = work.tile([128, 80], F32, tag="h_gq")
            Iq = work.tile([128, 80], F32, tag="h_iq")
            f_ap = fsc_s[:, i:i+1]
            nc.vector.tensor_scalar(out=Gq[:, 0:40], in0=BF[i][:, 168:208],
                                    scalar1=f_ap, scalar2=-1.0, op0=ALU.mult,
                                    op1=ALU.mult)
            nc.gpsimd.tensor_scalar(out=Gq[:, 40:80], in0=BF[i][:, 40:80],
                                    scalar1=f_ap, scalar2=None, op0=ALU.mult)
            nc.vector.tensor_scalar(out=Iq[:, 0:40], in0=BF[i][:, 128:168],
                                    scalar1=f_ap, scalar2=-1.0, op0=ALU.mult,
                                    op1=ALU.mult)
            nc.gpsimd.tensor_scalar(out=Iq[:, 40:80], in0=BF[i][:, 0:40],
                                    scalar1=f_ap, scalar2=None, op0=ALU.mult)
            GR3 = Gq[:, 0:40].rearrange("p (o r) -> p o r", o=1
                                        ).broadcast_to([128, 40, 40])
            GI3 = Gq[:, 40:80].rearrange("p (o r) -> p o r", o=1
                                         ).broadcast_to([128, 40, 40])
            IR3 = Iq[:, 0:40].rearrange("p (t o) -> p t o", o=1
                                        ).broadcast_to([128, 40, 40])
            II3 = Iq[:, 40:80].rearrange("p (t o) -> p t o", o=1
                                         ).broadcast_to([128, 40, 40])
            qr = work.tile([128, L16], F32, tag="h_qr")
            qi = work.tile([128, L16], F32, tag="h_qi")
            ta = work.tile([128, L16], F32, tag="h_ta")
            tb = work.tile([128, L16], F32, tag="h_tb")
            qr3 = qr[:].rearrange("p (t r) -> p t r", t=40)
            qi3 = qi[:].rearrange("p (t r) -> p t r", t=40)
            ta3 = ta[:].rearrange("p (t r) -> p t r", t=40)
            tb3 = tb[:].rearrange("p (t r) -> p t r", t=40)
            nc.vector.tensor_tensor(out=qr3, in0=GR3, in1=IR3, op=ALU.mult)
            nc.gpsimd.tensor_tensor(out=ta3, in0=GI3, in1=II3, op=ALU.mult)
            nc.gpsimd.tensor_tensor(out=qi3, in0=GI3, in1=IR3, op=ALU.mult)
            nc.vector.tensor_tensor(out=tb3, in0=GR3, in1=II3, op=ALU.mult)
            nc.vector.tensor_tensor(out=qr[:], in0=qr[:], in1=ta[:],
                                    op=ALU.subtract)
            nc.gpsimd.tensor_tensor(out=qi[:], in0=qi[:], in1=tb[:],
                                    op=ALU.add)
            # H rows: hr = qr*wr - qi*wi ; hi_stored = -(qr*wi + qi*wr)
            nc.vector.tensor_tensor(out=ta[:], in0=qr[:], in1=wrep_r[:],
                                    op=ALU.mult)
            nc.gpsimd.tensor_tensor(out=tb[:], in0=qi[:], in1=wrep_i[:],
                                    op=ALU.mult)
            nc.vector.tensor_tensor(out=sre[:, 0:L16], in0=ta[:], in1=tb[:],
                                    op=ALU.subtract)
            nc.gpsimd.tensor_tensor(out=ta[:], in0=qr[:], in1=wrep_i[:],
                                    op=ALU.mult)
            nc.vector.tensor_tensor(out=tb[:], in0=qi[:], in1=wrep_r[:],
                                    op=ALU.mult)
            nc.vector.scalar_tensor_tensor(out=sim[:, 0:L16], in0=ta[:],
                                           scalar=-1.0, in1=tb[:],
                                           op0=ALU.mult, op1=ALU.subtract)
            junk = work.tile([128, L16], BF16, tag="h_junk")
            nc.vector.scalar_tensor_tensor(
                out=junk[:], in0=sre[:, 0:L16], scalar=1.0, in1=drep[:],
                op0=ALU.mult, op1=ALU.mult, accum_out=vsum[:, i:i+1])
            junk2 = work.tile([128, L16], BF16, tag="h_junk2")
            nc.vector.scalar_tensor_tensor(
                out=junk2[:], in0=sim[:, 0:L16], scalar=1.0, in1=drep[:],
                op0=ALU.mult, op1=ALU.mult, accum_out=vsum[:, NB+i:NB+i+1])
            nc.sync.dma_start(scr["htdram"][128*i:128*(i+1), :], sre[:])
            nc.sync.dma_start(scr["htdram"][N+128*i:N+128*(i+1), :], sim[:])
            if i % 4 == 3:
                gram_round(i // 4, 8)
        gram_round(4, 4)

        vsq = work.tile([128, 2 * NB], F32, tag="vsq")
        nc.vector.tensor_tensor(out=vsq[:], in0=vsum[:], in1=vsum[:],
                                op=ALU.mult)
        vred = work.tile([128, 1], F32, tag="vred")
        nc.vector.tensor_reduce(vred[:], vsq[:], axis=AXX, op=ALU.add)
        nc.gpsimd.partition_all_reduce(vred[:], vred[:], 128,
                                       bass_isa.ReduceOp.add)
        nc.scalar.activation(lam[:], vred[:], AF.Sqrt)
        nc.vector.tensor_scalar(out=lam[:], in0=lam[:], scalar1=float(alpha),
                                scalar2=None, op0=ALU.mult)

    # ---------------- P8: scaled SPD block solve ----------------
    gr_pool = ctx.enter_context(tc.tile_pool(name="gr", bufs=1))
    GR = [gr_pool.tile([128, (LB - i) * 128], BF16, tag=f"gr{i}",
                       name=f"gr{i}") for i in range(LB)]
    BF2 = [gr_pool.tile([128, 1], BF16, tag=f"b2_{l}", name=f"b2_{l}")
           for l in range(LB)]
    ys = late.tile([128, LB], F32)
    with (
        tc.tile_pool(name="s_work", bufs=2) as work,
        tc.tile_pool(name="s_nw", bufs=1) as nwork,
        tc.tile_pool(name="s_pmm", bufs=1, space="PSUM") as pmm,
        tc.tile_pool(name="s_pup", bufs=1, space="PSUM") as pup,
        tc.tile_pool(name="s_pmisc", bufs=1, space="PSUM") as pmisc,
    ):
        # jacobi scaling vector from Gram diagonal
        gdiag = work.tile([128, LB], F32, tag="gdiag")
        for l in range(LB):
            t128 = work.tile([128, 128], F32, tag="gd_t")
            nc.vector.scalar_tensor_tensor(
                out=t128[:], in0=GA[l][:, 128*l:128*(l+1)], scalar=1.0,
                in1=id_s[:], op0=ALU.mult, op1=ALU.mult,
                accum_out=gdiag[:, l:l+1])
        nc.vector.tensor_scalar(out=gdiag[:], in0=gdiag[:], scalar1=lam[:],
                                scalar2=None, op0=ALU.add)
        nc.scalar.activation(st_[:], gdiag[:], AF.Sqrt)
        nc.vector.reciprocal(st_[:], st_[:])
        ps_ = pmisc.tile([LB, 128], F32, tag="s_ps")
        nc.tensor.matmul(ps_[:], st_[:], id_s[:], start=True, stop=True)
        s13 = work.tile([LB, 128], F32, tag="s13")
        nc.vector.tensor_copy(s13[:], ps_[:])
        nc.sync.dma_start(bass.AP(scr["srowdram"], 0, [[1, LPAD]]), s13[:])
        srow = work.tile([1, LPAD], F32, tag="srow")
        nc.sync.dma_start(srow[:], bass.AP(scr["srowdram"], 0, [[1, LPAD]]))
        nc.gpsimd.partition_broadcast(srep[:], srow[:])

        onesb = work.tile([128, 1], BF16, tag="onesb")
        nc.vector.memset(onesb[:], 1.0)
        for i in range(LB):
            for j in range(i, LB):
                nc.vector.scalar_tensor_tensor(
                    out=GR[i][:, (j-i)*128:(j-i)*128+128],
                    in0=GA[i][:, 128*j:128*(j+1)], scalar=st_[:, i:i+1],
                    in1=srep[:, 128*j:128*(j+1)], op0=ALU.mult, op1=ALU.mult)
            nc.vector.copy_predicated(GR[i][:, 0:128], idu_s[:],
                                      onesb[:].broadcast_to([128, 128]))
        dsc = work.tile([128, LB], F32, tag="dsc")
        nc.vector.tensor_tensor(out=dsc[:], in0=dvec[:], in1=st_[:],
                                op=ALU.mult)
        for l in range(LB):
            nc.gpsimd.tensor_copy(BF2[l][:], dsc[:, l:l+1])

        pending = []

        def drain(n):
            for _ in range(min(n, len(pending))):
                pending.pop(0)()

        for step in _newton_real_steps(nc, nwork, pmm, pmisc, GR[0][:, 0:128],
                                       consts, NEWTON_SPD):
            step()
        for k in range(LB):
            nr = LB - 1 - k
            if nr > 0:
                # LT row = V_k @ (pivot row right of diag), wide
                LTrow = work.tile([128, (LB - 1) * 128], BF16, tag="lt_row")
                Wk = nr * 128
                for c in range(0, Wk, 512):
                    w = min(512, Wk - c)
                    pl = pup.tile([128, 512], F32, tag="s_pl")
                    nc.tensor.matmul(pl[:, 0:w], GR[k][:, 0:128],
                                     GR[k][:, 128+c:128+c+w],
                                     start=True, stop=True)
                    nc.scalar.copy(LTrow[:, c:c+w], pl[:, 0:w])
            for i in range(k + 1, LB):
                lt = LTrow[:, (i-k-1)*128:(i-k)*128]
                pb = pmm.tile([128, 1], F32, tag="s_pb")
                nc.tensor.matmul(pb[:], lt, BF2[k][:], start=True, stop=True)
                nc.vector.tensor_tensor(out=BF2[i][:], in0=BF2[i][:],
                                        in1=pb[:], op=ALU.subtract)
                Wi = (LB - i) * 128
                for c in range(0, Wi, 512):
                    w = min(512, Wi - c)
                    pu = pup.tile([128, 512], F32, tag="s_pu")
                    nc.tensor.matmul(pu[:, 0:w], lt,
                                     GR[k][:, (i-k)*128+c:(i-k)*128+c+w],
                                     start=True, stop=True)
                    mux.i += 1
                    if mux.i & 1:
                        nc.vector.tensor_tensor(out=GR[i][:, c:c+w],
                                                in0=GR[i][:, c:c+w],
                                                in1=pu[:, 0:w],
                                                op=ALU.subtract)
                    else:
                        aptmp = work.tile([128, 512], BF16, tag="ap_tmp8")
                        nc.scalar.copy(aptmp[:, 0:w], pu[:, 0:w])
                        nc.gpsimd.tensor_tensor(out=GR[i][:, c:c+w],
                                                in0=GR[i][:, c:c+w],
                                                in1=aptmp[:, 0:w],
                                                op=ALU.subtract)
                if i == k + 1:
                    pending = list(_newton_real_steps(
                        nc, nwork, pmm, pmisc, GR[i][:, 0:128], consts,
                        NEWTON_SPD))
                    drain(3)
                else:
                    drain(3)
            drain(len(pending))

        # backward substitution
        for k in range(LB - 1, -1, -1):
            P1a = pmm.tile([128, 1], F32, tag="s_pb")
            nc.tensor.matmul(P1a[:], consts["nIb"][:], BF2[k][:],
                             start=True, stop=(k == LB - 1))
            for j in range(k + 1, LB):
                utt = work.tile([128, 128], BF16, tag="s_utt")
                nc.sync.dma_start_transpose(
                    utt[:], GR[k][:, (j-k)*128:(j-k+1)*128])
                nc.tensor.matmul(P1a[:], utt[:], BF2[j][:],
                                 start=False, stop=(j == LB - 1))
            W2 = work.tile([128, 1], BF16, tag="s_W2")
            nc.vector.tensor_copy(W2[:], P1a[:])
            PS = pmm.tile([128, 1], F32, tag="s_pb")
            nc.tensor.matmul(PS[:], GR[k][:, 0:128], W2[:],
                             start=True, stop=True)
            nc.vector.tensor_scalar(out=BF2[k][:], in0=PS[:], scalar1=-1.0,
                                    scalar2=None, op0=ALU.mult)
        for l in range(LB):
            nc.gpsimd.tensor_copy(ys[:, l:l+1], BF2[l][:])
        nc.vector.tensor_tensor(out=ys[:], in0=ys[:], in1=st_[:], op=ALU.mult)
        psy = pmisc.tile([LB, 128], F32, tag="y_ps")
        nc.tensor.matmul(psy[:], ys[:], id_s[:], start=True, stop=True)
        y13 = work.tile([LB, 128], F32, tag="y13")
        nc.vector.tensor_copy(y13[:], psy[:])
        nc.sync.dma_start(bass.AP(scr["yrowdram"], 0, [[1, LPAD]]), y13[:])
        yrow = work.tile([1, LPAD], F32, tag="yrow")
        nc.sync.dma_start(yrow[:], bass.AP(scr["yrowdram"], 0, [[1, LPAD]]))
        nc.gpsimd.partition_broadcast(yrep[:], yrow[:])

    # ---------------- P9: chi = Ht y ----------------
    with tc.tile_pool(name="p9_work", bufs=3) as work:
        chi = late.tile([128, 2 * NB], F32)
        for ch in range(2 * NB):
            htc = work.tile([128, LPAD], BF16, tag="c_htc")
            nc.sync.dma_start(htc[:], scr["htdram"][128*ch:128*(ch+1), :])
            junk = work.tile([128, LPAD], BF16, tag="c_junk")
            nc.vector.scalar_tensor_tensor(
                out=junk[:], in0=htc[:], scalar=1.0, in1=yrep[:],
                op0=ALU.mult, op1=ALU.mult, accum_out=chi[:, ch:ch+1])
        nc.sync.dma_start(bass.AP(out_chi, 0, [[1, 128], [128, 2 * NB]]),
                          chi[:])
    ctx.close()


_CACHED = {}


def kernel(epsilon_r_iter, chi_iter, total_power, alpha, grid_x, grid_y,
           direct_field, incident_field, G_freespace, G_freespace_scaled,
           sensor_links):
    eps = np.asarray(epsilon_r_iter)
    chi_it = np.asarray(chi_iter)
    tp = np.asarray(total_power, dtype=np.float32)
    alpha_f = float(np.asarray(alpha))
    gx = np.asarray(grid_x, dtype=np.float32)
    gy = np.asarray(grid_y, dtype=np.float32)
    df = np.asarray(direct_field)
    einc = np.asarray(incident_field)
    gfs = np.asarray(G_freespace)
    gsc = np.asarray(G_freespace_scaled)
    links = np.asarray(sensor_links)

    # this kernel assumes the canonical uniform link set (t-major, r != t)
    expect = np.array([[t, r] for t in range(TX) for r in range(RX) if r != t],
                      dtype=np.int32)
    assert links.shape == expect.shape and np.array_equal(links, expect), \
        "kernel specialized for the canonical sensor_links layout"

    x = gx.T.reshape(N).astype(np.float32)
    y = gy.T.reshape(N).astype(np.float32)
    scat = np.real(eps.T.reshape(N)).astype(np.float32)

    geomS = np.stack([np.ones(N, np.float32), -2.0*x, -2.0*y,
                      (x*x + y*y)]).astype(np.float32)
    geomR = np.stack([(x*x + y*y), x, y,
                      np.ones(N, np.float32)]).astype(np.float32)
    scat_t = scat.reshape(NB, 128).T.copy()

    bpack = np.zeros((N, RW), np.float32)
    bpack[:, 0:40] = -einc.real; bpack[:, 40:80] = -gfs.real
    bpack[:, 128:168] = -einc.imag; bpack[:, 168:208] = -gfs.imag
    gscT = np.concatenate([gsc.real.T, gsc.imag.T], axis=1).astype(np.float32)
    dfpack = np.concatenate([df.real, df.imag], axis=1).astype(np.float32)

    # total_power [RX-1, TX] -> [40, 40] with zeros on the diagonal
    tp40 = np.zeros((40, 40), np.float32)
    for t in range(TX):
        rs = [r for r in range(RX) if r != t]
        tp40[rs, t] = tp[:, t]

    key = alpha_f
    if key not in _CACHED:
        _CACHED[key] = build_program(alpha_f)
    nc = _CACHED[key]

    id128 = np.eye(128, dtype=np.float32)
    im = {
        "geomS": geomS, "geomR": geomR, "scat_t": scat_t, "bpack": bpack,
        "gscT": gscT, "dfpack": dfpack, "tp40": tp40,
        "id128": id128, "idu8": id128.astype(np.uint8),
    }
    import os as _os
    _tr = _os.environ.get("KTRACE", "0") == "1"
    res = run_bass_kernel_spmd(nc, [im] * 8, core_ids=list(range(8)),
                               trace=_tr)
    out = res.results[0]
    _CACHED["last"] = (res, out)

    chi = np.asarray(out["out_chi"], dtype=np.float32)
    dchi_r = chi[:N].reshape(M, M).T
    dchi_i = chi[N:].reshape(M, M).T
    chi_new = (chi_it + (dchi_r + 1j * dchi_i)).astype(np.complex64)
    return chi_new + 1.0, chi_new


# revision 15
# speedup vs baseline: 1.0793x; 1.0793x over previous
"""DRIM layer (distorted Rytov inverse-scattering iteration) on Trainium2.

Optimized single-core program replicated SPMD on 8 cores.  Key design:
  - all bulk matrix state (Z, factors, rhs, H) stored bf16 in SBUF/DRAM;
    fp32 PSUM accumulation everywhere (validated end-to-end ~1e-3)
  - elementwise work split across DVE (vector) and Pool (gpsimd) engines
  - complex products via plane-swapped (-im|re) rhs copies so each complex
    matmul is 2 wide PSUM-accumulating matmuls, one combine op
  - sin/cos range reduction via one fused (x+pi mod 2pi) tensor_scalar
  - activation-table churn avoided (two-pass Z build: sqrt pass, sin pass)
  - Newton block inversions emitted interleaved with trailing updates
  - pivot-row transposes via XBAR DMA-transpose loads (no PE transposes)
  - Gram accumulated over 4-row-chunk quads in PSUM
"""
import math
import numpy as np

import concourse.bass as bass
import concourse.bacc as bacc
import concourse.bass_isa as bass_isa
import concourse.mybir as mybir
import concourse.tile as tile
from concourse.bass_utils import run_bass_kernel_spmd

F32 = mybir.dt.float32
F32R = mybir.dt.float32r
BF16 = mybir.dt.bfloat16
U8 = mybir.dt.uint8
I32 = mybir.dt.int32
AF = mybir.ActivationFunctionType
ALU = mybir.AluOpType
AXX = mybir.AxisListType.X

M = 48
N = M * M
NB = N // 128               # 18
TX = RX = 40
L16 = 1600                  # 40x40 links incl. zero-weighted diagonal
LPAD = 1664
LB = LPAD // 128            # 13
RW = 256
DOI = 3.0
WL = 0.125
K0 = 2.0 * math.pi / WL
IMP = 120.0 * math.pi
GRID_LEN = DOI / M
GRID_RADIUS = math.sqrt(GRID_LEN ** 2 / math.pi)
NOISE = 1e-6

def _j1s(x):
    t2 = (x / 3.0) ** 2
    return x * (0.5 - 0.56249985*t2 + 0.21093573*t2**2 - 0.03954289*t2**3
                + 0.00443319*t2**4 - 0.00031761*t2**5 + 0.00001109*t2**6)

def _y1s(x):
    t2 = (x / 3.0) ** 2
    p = (-0.6366198 + 0.2212091*t2 + 2.1682709*t2**2 - 1.3164827*t2**3
         + 0.3123951*t2**4 - 0.0400976*t2**5 + 0.0027873*t2**6)
    return ((2.0/math.pi) * x * math.log(0.5*x) * _j1s(x) + p) / x

X0C = K0 * GRID_RADIUS
GRID_AREA = 4.0*math.pi*GRID_RADIUS/(2.0*K0) * _j1s(X0C)
C1 = -IMP * math.pi * GRID_RADIUS / 2.0
C2 = _j1s(X0C)
C3R, C3I = _j1s(X0C), _y1s(X0C)
C1C2 = C1 * C2
ZD_RE = C1 * C3R
ZD_IM_C = C1 * C3I
SA = GRID_AREA * K0 * K0
TWO_PI = 2.0 * math.pi
INV_2PI = 1.0 / TWO_PI
LOG10E20 = 20.0 * math.log10(math.e)
CADD = 10.0 * math.log10(WL * WL / (4.0 * math.pi * IMP) / 1e-3)
C20L = 20.0 / math.log(10.0)

F0C = [0.79788456, -0.00000077, -0.00552740, -0.00009512]
THC = [-0.78539816, -0.04166397, -0.00003954, 0.00262573]
F0CS = [c * (3.0 ** k) * C1C2 for k, c in enumerate(F0C)]
THCS = [c * (3.0 ** k) for k, c in enumerate(THC)]

NEWTON_Z = 17
NEWTON_SPD = 14


class Mux:
    """Alternate elementwise ops between DVE (vector) and Pool (gpsimd)."""
    def __init__(self, nc):
        self.nc = nc
        self.i = 0

    def eng(self):
        self.i += 1
        return self.nc.vector if (self.i & 1) else self.nc.gpsimd


def build_program(alpha):
    nc = bacc.Bacc("TRN2", target_bir_lowering=False, num_devices=8)
    din = {}
    def inp(name, shape, dtype=F32):
        din[name] = nc.dram_tensor(name, shape, dtype, kind="ExternalInput")
    inp("geomS", [4, N]); inp("geomR", [4, N]); inp("scat_t", [128, NB])
    inp("bpack", [N, RW]); inp("gscT", [N, 80]); inp("dfpack", [40, 80])
    inp("tp40", [40, 40]); inp("id128", [128, 128]); inp("idu8", [128, 128], U8)
    out_chi = nc.dram_tensor("out_chi", [2 * N], F32, kind="ExternalOutput")
    xdbg = nc.dram_tensor("xdbg", [N, RW], BF16, kind="ExternalOutput")
    tfdbg = nc.dram_tensor("tfdbg", [40, 80], F32, kind="ExternalOutput")
    scr = {}
    scr["utdram"] = nc.dram_tensor("utdram", [N, 2 * N], BF16, kind="Internal")
    scr["htdram"] = nc.dram_tensor("htdram", [2 * N, LPAD], BF16, kind="Internal")
    scr["sdram"] = nc.dram_tensor("sdram", [L16], F32, kind="Internal")
    scr["wrdram"] = nc.dram_tensor("wrdram", [L16], F32, kind="Internal")
    scr["widram"] = nc.dram_tensor("widram", [L16], F32, kind="Internal")
    scr["srowdram"] = nc.dram_tensor("srowdram", [LPAD], F32, kind="Internal")
    scr["yrowdram"] = nc.dram_tensor("yrowdram", [LPAD], F32, kind="Internal")

    with tile.TileContext(nc) as tc:
        _body(nc, tc, din, out_chi, xdbg, tfdbg, scr, alpha)
    nc.compile()
    return nc


def _newton_scale(nc, work, pmisc, m, tag):
    """a = 1/(max rowsum)^2 of |m| (m symmetric) -> [128,1] f32 AP."""
    cs = work.tile([128, 1], F32, tag=f"nwcs_{tag}")
    nc.vector.tensor_reduce(cs[:], m[:], axis=AXX, op=ALU.add)
    nc.gpsimd.partition_all_reduce(cs[:], cs[:], 128, bass_isa.ReduceOp.max)
    a = work.tile([128, 1], F32, tag=f"nwa_{tag}")
    nc.vector.tensor_tensor(out=a[:], in0=cs[:], in1=cs[:], op=ALU.mult)
    nc.vector.reciprocal(a[:], a[:])
    return a


def _newton_cplx_steps(nc, work, pmm, pmisc, Dap, consts, iters):
    """Generator of emission closures for one complex Newton inversion.

    Dap: [128,256] bf16 (re|im) block, symmetric; V is written back to Dap.
    """
    st = {}

    def prologue():
        m1 = work.tile([128, 128], F32, tag="nw_m1")
        m2 = work.tile([128, 128], F32, tag="nw_m2")
        nc.scalar.activation(m1[:], Dap[:, 0:128], AF.Abs)
        nc.scalar.activation(m2[:], Dap[:, 128:256], AF.Abs)
        nc.vector.tensor_tensor(out=m1[:], in0=m1[:], in1=m2[:], op=ALU.max)
        a = _newton_scale(nc, work, pmisc, m1, "c")
        X = work.tile([128, RW], BF16, tag="nw_X")
        XB = work.tile([128, RW], BF16, tag="nw_XB")
        nc.vector.tensor_scalar(out=X[:, 0:128], in0=Dap[:, 0:128],
                                scalar1=a[:], scalar2=None, op0=ALU.mult)
        nc.vector.tensor_scalar(out=X[:, 128:256], in0=Dap[:, 128:256],
                                scalar1=a[:], scalar2=-1.0, op0=ALU.mult,
                                op1=ALU.mult)
        nc.gpsimd.tensor_scalar(out=XB[:, 0:128], in0=Dap[:, 128:256],
                                scalar1=a[:], scalar2=None, op0=ALU.mult)
        nc.gpsimd.tensor_scalar(out=XB[:, 128:256], in0=Dap[:, 0:128],
                                scalar1=a[:], scalar2=None, op0=ALU.mult)
        st["X"], st["XB"] = X, XB

    yield prologue

    def one_iter():
        X, XB = st["X"], st["XB"]
        PP = pmm.tile([128, RW], F32, tag="nw_PP")
        nc.tensor.matmul(PP[:], Dap[:, 0:128], X[:], start=True, stop=False)
        nc.tensor.matmul(PP[:], Dap[:, 128:256], XB[:], start=False, stop=True)
        R = work.tile([128, RW], BF16, tag="nw_R")
        RB = work.tile([128, RW], BF16, tag="nw_RB")
        nc.vector.tensor_tensor(out=R[:], in0=consts["Ip"][:], in1=PP[:],
                                op=ALU.subtract)
        nc.vector.tensor_scalar(out=RB[:, 0:128], in0=R[:, 128:256],
                                scalar1=-1.0, scalar2=None, op0=ALU.mult)
        nc.vector.tensor_copy(RB[:, 128:256], R[:, 0:128])
        QQ = pmm.tile([128, RW], F32, tag="nw_QQ")
        nc.tensor.matmul(QQ[:], X[:, 0:128], R[:], start=True, stop=False)
        nc.tensor.matmul(QQ[:], X[:, 128:256], RB[:], start=False, stop=True)
        nc.vector.tensor_tensor(out=X[:], in0=X[:], in1=QQ[:], op=ALU.add)
        nc.vector.tensor_scalar(out=XB[:, 0:128], in0=X[:, 128:256],
                                scalar1=-1.0, scalar2=None, op0=ALU.mult)
        nc.vector.tensor_copy(XB[:, 128:256], X[:, 0:128])

    for _ in range(iters):
        yield one_iter

    def final():
        nc.vector.tensor_copy(Dap[:], st["X"][:])

    yield final


def _newton_real_steps(nc, work, pmm, pmisc, Dap, consts, iters):
    """Same for a real symmetric [128,128] bf16 block; V written to Dap."""
    st = {}

    def prologue():
        m1 = work.tile([128, 128], F32, tag="nw2_m1")
        nc.scalar.activation(m1[:], Dap[:], AF.Abs)
        a = _newton_scale(nc, work, pmisc, m1, "r")
        X = work.tile([128, 128], BF16, tag="nw2_X")
        nc.vector.tensor_scalar(out=X[:], in0=Dap[:], scalar1=a[:],
                                scalar2=None, op0=ALU.mult)
        st["X"] = X

    yield prologue

    def one_iter():
        X = st["X"]
        PP = pmm.tile([128, 128], F32, tag="nw2_PP")
        nc.tensor.matmul(PP[:], Dap[:], X[:], start=True, stop=True)
        R = work.tile([128, 128], BF16, tag="nw2_R")
        nc.vector.tensor_tensor(out=R[:], in0=consts["Ib"][:], in1=PP[:],
                                op=ALU.subtract)
        QQ = pmm.tile([128, 128], F32, tag="nw2_QQ")
        nc.tensor.matmul(QQ[:], X[:], R[:], start=True, stop=True)
        nc.vector.tensor_tensor(out=X[:], in0=X[:], in1=QQ[:], op=ALU.add)

    for _ in range(iters):
        yield one_iter

    def final():
        nc.vector.tensor_copy(Dap[:], st["X"][:])

    yield final


def _body(nc, tc, din, out_chi, xdbg, tfdbg, scr, alpha):
    import contextlib
    ctx = contextlib.ExitStack()
    mux = Mux(nc)

    consts_pool = ctx.enter_context(tc.tile_pool(name="consts", bufs=1))
    id_s = consts_pool.tile([128, 128], F32)
    nc.sync.dma_start(id_s[:], din["id128"][:])
    idu_s = consts_pool.tile([128, 128], U8)
    nc.sync.dma_start(idu_s[:], din["idu8"][:])
    idb_s = consts_pool.tile([128, 128], BF16)      # +I bf16
    nc.vector.tensor_copy(idb_s[:], id_s[:])
    nidb_s = consts_pool.tile([128, 128], BF16)     # -I bf16
    nc.gpsimd.tensor_scalar(out=nidb_s[:], in0=id_s[:], scalar1=-1.0,
                            scalar2=None, op0=ALU.mult)
    Ip_s = consts_pool.tile([128, RW], BF16)        # (I|0)
    nc.vector.memset(Ip_s[:], 0.0)
    nc.vector.tensor_copy(Ip_s[:, 0:128], id_s[:])
    Iq_s = consts_pool.tile([128, RW], BF16)        # (0|I)
    nc.gpsimd.memset(Iq_s[:], 0.0)
    nc.gpsimd.tensor_copy(Iq_s[:, 128:256], id_s[:])
    scat_s = consts_pool.tile([128, NB], F32)
    nc.sync.dma_start(scat_s[:], din["scat_t"][:])

    zdi_s = consts_pool.tile([128, NB], BF16)
    fsc_s = consts_pool.tile([128, NB], F32)
    t0 = consts_pool.tile([128, NB], F32)
    nc.vector.tensor_scalar(out=t0[:], in0=scat_s[:], scalar1=-1.0,
                            scalar2=None, op0=ALU.add)
    nc.vector.reciprocal(t0[:], t0[:])
    nc.vector.tensor_scalar(out=fsc_s[:], in0=t0[:], scalar1=(IMP / K0),
                            scalar2=None, op0=ALU.mult)
    nc.vector.tensor_tensor(out=t0[:], in0=t0[:], in1=scat_s[:], op=ALU.mult)
    nc.vector.tensor_scalar(out=zdi_s[:], in0=t0[:], scalar1=-(IMP / K0),
                            scalar2=ZD_IM_C, op0=ALU.mult, op1=ALU.add)
    zdr_c = consts_pool.tile([128, 1], BF16)
    nc.vector.memset(zdr_c[:], float(ZD_RE))
    npi_c = consts_pool.tile([128, 1], F32)
    nc.vector.memset(npi_c[:], -math.pi)
    pi2_c = consts_pool.tile([128, 1], F32)
    nc.vector.memset(pi2_c[:], math.pi / 2.0)

    bf_pool = ctx.enter_context(tc.tile_pool(name="bf", bufs=1))
    BF = [bf_pool.tile([128, RW], BF16, tag=f"bf{i}", name=f"bf{i}")
          for i in range(NB)]

    consts = {"Ip": Ip_s, "Iq": Iq_s, "Ib": idb_s, "nIb": nidb_s}

    with tc.tile_pool(name="tri", bufs=1) as tri:
        ZR = [tri.tile([128, (NB - i) * RW], BF16, tag=f"zr{i}", name=f"zr{i}")
              for i in range(NB)]

        # ---------------- P1: Z build ----------------
        with (
            tc.tile_pool(name="zb_geom", bufs=2) as gpool,
            tc.tile_pool(name="zb_scr", bufs=1) as spool,
            tc.tile_pool(name="zb_work", bufs=2) as work,
            tc.tile_pool(name="zb_psum", bufs=3, space="PSUM") as pz,
        ):
            th_s = spool.tile([128, N], F32, name="th_s")
            amp_s = spool.tile([128, N], BF16, name="amp_s")
            for k in range(NB):
                r0 = 128 * k
                Wr = (NB - k) * 128
                gS0 = gpool.tile([4, 128], F32, tag="gS0", name="gS0")
                nc.sync.dma_start(gS0[:], din["geomS"][:, r0:r0+128])
                gS = gpool.tile([4, 128], F32R, tag="gS", name="gS")
                nc.vector.tensor_copy(gS[:], gS0[:])
                # pass A: distances, polynomials, amplitude, phase
                for c in range(0, Wr, 512):
                    w = min(512, Wr - c)
                    gR0 = work.tile([4, 512], F32, tag="gR0")
                    nc.sync.dma_start(gR0[:, 0:w], din["geomR"][:, r0+c:r0+c+w])
                    gR = work.tile([4, 512], F32R, tag="gR")
                    nc.vector.tensor_copy(gR[:, 0:w], gR0[:, 0:w])
                    pd = pz.tile([128, 512], F32, tag="zb_pd")
                    nc.tensor.matmul(pd[:, 0:w], gS[:], gR[:, 0:w],
                                     start=True, stop=True)
                    dsq = work.tile([128, 512], F32, tag="zb_dsq")
                    nc.vector.tensor_scalar(out=dsq[:, 0:w], in0=pd[:, 0:w],
                                            scalar1=0.002, scalar2=None,
                                            op0=ALU.max)
                    x = work.tile([128, 512], F32, tag="zb_x")
                    nc.scalar.activation(x[:, 0:w], dsq[:, 0:w], AF.Sqrt,
                                         scale=float(K0 * K0))
                    sp = work.tile([128, 512], F32, tag="zb_sp")
                    nc.vector.reciprocal(sp[:, 0:w], x[:, 0:w])
                    s2 = work.tile([128, 512], F32, tag="zb_s2")
                    nc.gpsimd.tensor_tensor(out=s2[:, 0:w], in0=sp[:, 0:w],
                                            in1=sp[:, 0:w], op=ALU.mult)
                    t1 = work.tile([128, 512], F32, tag="zb_t1")
                    nc.vector.tensor_scalar(out=t1[:, 0:w], in0=sp[:, 0:w],
                                            scalar1=THCS[1], scalar2=THCS[0],
                                            op0=ALU.mult, op1=ALU.add)
                    t2 = work.tile([128, 512], F32, tag="zb_t2")
                    nc.gpsimd.tensor_scalar(out=t2[:, 0:w], in0=sp[:, 0:w],
                                            scalar1=THCS[3], scalar2=THCS[2],
                                            op0=ALU.mult, op1=ALU.add)
                    nc.vector.tensor_tensor(out=t1[:, 0:w], in0=t1[:, 0:w],
                                            in1=x[:, 0:w], op=ALU.add)
                    nc.gpsimd.tensor_tensor(out=t2[:, 0:w], in0=t2[:, 0:w],
                                            in1=s2[:, 0:w], op=ALU.mult)
                    nc.vector.tensor_tensor(out=th_s[:, c:c+w], in0=t1[:, 0:w],
                                            in1=t2[:, 0:w], op=ALU.add)
                    u1 = work.tile([128, 512], F32, tag="zb_u1")
                    nc.gpsimd.tensor_scalar(out=u1[:, 0:w], in0=sp[:, 0:w],
                                            scalar1=F0CS[1], scalar2=F0CS[0],
                                            op0=ALU.mult, op1=ALU.add)
                    u2 = work.tile([128, 512], F32, tag="zb_u2")
                    nc.vector.tensor_scalar(out=u2[:, 0:w], in0=sp[:, 0:w],
                                            scalar1=F0CS[3], scalar2=F0CS[2],
                                            op0=ALU.mult, op1=ALU.add)
                    sqx = work.tile([128, 512], F32, tag="zb_sqx")
                    nc.scalar.activation(sqx[:, 0:w], sp[:, 0:w], AF.Sqrt)
                    nc.vector.tensor_tensor(out=u2[:, 0:w], in0=u2[:, 0:w],
                                            in1=s2[:, 0:w], op=ALU.mult)
                    nc.gpsimd.tensor_tensor(out=u1[:, 0:w], in0=u1[:, 0:w],
                                            in1=u2[:, 0:w], op=ALU.add)
                    nc.gpsimd.tensor_tensor(out=amp_s[:, c:c+w],
                                            in0=u1[:, 0:w], in1=sqx[:, 0:w],
                                            op=ALU.mult)
                # pass B: sines into ZR row (strided per-plane writes)
                for c in range(0, Wr, 512):
                    w = min(512, Wr - c)
                    nblk = w // 128
                    u = work.tile([128, 512], F32, tag="zb_u")
                    nc.vector.tensor_scalar(out=u[:, 0:w], in0=th_s[:, c:c+w],
                                            scalar1=INV_2PI, scalar2=None,
                                            op0=ALU.mult)
                    ki = work.tile([128, 512], I32, tag="zb_ki")
                    nc.vector.tensor_copy(ki[:, 0:w], u[:, 0:w])
                    mf = work.tile([128, 512], F32, tag="zb_mf")
                    nc.vector.tensor_copy(mf[:, 0:w], ki[:, 0:w])
                    sa = work.tile([128, 512], F32, tag="zb_sa")
                    nc.vector.scalar_tensor_tensor(
                        out=sa[:, 0:w], in0=mf[:, 0:w], scalar=-TWO_PI,
                        in1=th_s[:, c:c+w], op0=ALU.mult, op1=ALU.add)
                    sinr = work.tile([128, 512], F32, tag="zb_sin")
                    nc.scalar.activation(sinr[:, 0:w], sa[:, 0:w], AF.Sin)
                    u2 = work.tile([128, 512], F32, tag="zb_u2")
                    nc.gpsimd.tensor_scalar(out=u2[:, 0:w], in0=u[:, 0:w],
                                            scalar1=0.25, scalar2=None,
                                            op0=ALU.add)
                    ki2 = work.tile([128, 512], I32, tag="zb_ki2")
                    nc.vector.tensor_copy(ki2[:, 0:w], u2[:, 0:w])
                    mf2 = work.tile([128, 512], F32, tag="zb_mf2")
                    nc.vector.tensor_copy(mf2[:, 0:w], ki2[:, 0:w])
                    sa2 = work.tile([128, 512], F32, tag="zb_sa2")
                    nc.vector.scalar_tensor_tensor(
                        out=sa2[:, 0:w], in0=mf2[:, 0:w], scalar=-TWO_PI,
                        in1=th_s[:, c:c+w], op0=ALU.mult, op1=ALU.add)
                    cosr = work.tile([128, 512], F32, tag="zb_cos")
                    nc.scalar.activation(cosr[:, 0:w], sa2[:, 0:w], AF.Sin,
                                         bias=pi2_c[:])
                    zr3 = ZR[k][:, 2*c:2*c+nblk*RW].rearrange(
                        "p (n t) -> p n t", t=RW)
                    s3 = sinr[:, 0:w].rearrange("p (n t) -> p n t", t=128)
                    c3 = cosr[:, 0:w].rearrange("p (n t) -> p n t", t=128)
                    a3 = amp_s[:, c:c+w].rearrange("p (n t) -> p n t", t=128)
                    nc.gpsimd.tensor_tensor(out=zr3[:, :, 0:128], in0=c3,
                                            in1=a3, op=ALU.mult)
                    nc.vector.tensor_tensor(out=zr3[:, :, 128:256], in0=s3,
                                            in1=a3, op=ALU.mult)
                # diagonal overrides
                nc.vector.copy_predicated(
                    ZR[k][:, 0:128], idu_s[:],
                    zdr_c[:].broadcast_to([128, 128]))
                nc.vector.copy_predicated(
                    ZR[k][:, 128:256], idu_s[:],
                    zdi_s[:, k:k+1].broadcast_to([128, 128]))

        # ---------------- P2: block LDL^T ----------------
        with (
            tc.tile_pool(name="lu_zb", bufs=1) as zbpool,
            tc.tile_pool(name="lu_work", bufs=2) as work,
            tc.tile_pool(name="lu_nw", bufs=1) as nwork,
            tc.tile_pool(name="lu_pmm", bufs=1, space="PSUM") as pmm,
            tc.tile_pool(name="lu_pup", bufs=3, space="PSUM") as pup,
        ):
            pmisc = None
            ZB = zbpool.tile([128, (NB - 1) * RW], BF16, name="zbswap")
            BFB = zbpool.tile([128, RW], BF16, name="bfbswap")
            ldtmp = work.tile([128, RW], F32, tag="ldtmp")
            for i in range(NB):
                nc.sync.dma_start(ldtmp[:], din["bpack"][128*i:128*(i+1), :])
                nc.vector.tensor_copy(BF[i][:], ldtmp[:])
                ldtmp = work.tile([128, RW], F32, tag="ldtmp")

            pending = []

            def drain(n):
                for _ in range(min(n, len(pending))):
                    pending.pop(0)()

            for step in _newton_cplx_steps(nc, nwork, pmm, pmisc,
                                           ZR[0][:, 0:RW], consts, NEWTON_Z):
                step()

            for k in range(NB):
                nr = NB - 1 - k      # trailing rows
                if nr > 0:
                    # swapped pivot row (-im|re) for blocks k+1..17
                    zb3 = ZB[:, 0:nr*RW].rearrange("p (n t) -> p n t", t=RW)
                    zr3 = ZR[k][:, RW:(nr+1)*RW].rearrange(
                        "p (n t) -> p n t", t=RW)
                    nc.vector.tensor_scalar(out=zb3[:, :, 0:128],
                                            in0=zr3[:, :, 128:256],
                                            scalar1=-1.0, scalar2=None,
                                            op0=ALU.mult)
                    nc.gpsimd.tensor_copy(zb3[:, :, 128:256], zr3[:, :, 0:128])
                    # swapped pivot rhs
                    nc.vector.tensor_scalar(out=BFB[:, 0:128],
                                            in0=BF[k][:, 128:256],
                                            scalar1=-1.0, scalar2=None,
                                            op0=ALU.mult)
                    nc.gpsimd.tensor_copy(BFB[:, 128:256], BF[k][:, 0:128])
                    # store pivot row for backsolve (transposed on load)
                    nc.sync.dma_start(
                        scr["utdram"][128*k:128*(k+1), RW*(k+1):RW*NB],
                        ZR[k][:, RW:(nr+1)*RW])
                for i in range(k + 1, NB):
                    off = (i - k) * RW
                    zoff = (i - k - 1) * RW
                    PL = pmm.tile([128, RW], F32, tag="lu_PL")
                    nc.tensor.matmul(PL[:], ZR[k][:, 0:128],
                                     ZR[k][:, off:off+RW], start=True,
                                     stop=False)
                    nc.tensor.matmul(PL[:], ZR[k][:, 128:256],
                                     ZB[:, zoff:zoff+RW], start=False,
                                     stop=True)
                    LT = work.tile([128, RW], BF16, tag="lu_LT")
                    nc.scalar.copy(LT[:], PL[:])
                    # rhs update
                    PBf = pmm.tile([128, RW], F32, tag="lu_PBf")
                    nc.tensor.matmul(PBf[:], LT[:, 0:128], BF[k][:],
                                     start=True, stop=False)
                    nc.tensor.matmul(PBf[:], LT[:, 128:256], BFB[:],
                                     start=False, stop=True)
                    nc.vector.tensor_tensor(out=BF[i][:], in0=BF[i][:],
                                            in1=PBf[:], op=ALU.subtract)
                    # trailing row update, 512-wide chunks
                    Wi = (NB - i) * RW
                    for c in range(0, Wi, 512):
                        w = min(512, Wi - c)
                        PU = pup.tile([128, 512], F32, tag="lu_PU")
                        nc.tensor.matmul(PU[:, 0:w], LT[:, 0:128],
                                         ZR[k][:, off+c:off+c+w],
                                         start=True, stop=False)
                        nc.tensor.matmul(PU[:, 0:w], LT[:, 128:256],
                                         ZB[:, zoff+c:zoff+c+w],
                                         start=False, stop=True)
                        mux.i += 1
                        if mux.i % 3 == 0:
                            nc.vector.tensor_tensor(out=ZR[i][:, c:c+w],
                                                    in0=ZR[i][:, c:c+w],
                                                    in1=PU[:, 0:w],
                                                    op=ALU.subtract)
                        else:
                            aptmp = work.tile([128, 512], BF16, tag="ap_tmp")
                            nc.scalar.copy(aptmp[:, 0:w], PU[:, 0:w])
                            nc.gpsimd.tensor_tensor(out=ZR[i][:, c:c+w],
                                                    in0=ZR[i][:, c:c+w],
                                                    in1=aptmp[:, 0:w],
                                                    op=ALU.subtract)
                    if i == k + 1:
                        pending = list(_newton_cplx_steps(
                            nc, nwork, pmm, pmisc, ZR[i][:, 0:RW], consts,
                            NEWTON_Z))
                        drain(2)
                    else:
                        drain(2)
                drain(len(pending))

        # ---------------- P3: back-substitution ----------------
        with (
            tc.tile_pool(name="bs_pre", bufs=1) as pre,
            tc.tile_pool(name="bs_work", bufs=3) as work,
            tc.tile_pool(name="bs_pacc", bufs=2, space="PSUM") as pacc,
            tc.tile_pool(name="bs_pmm", bufs=2, space="PSUM") as pmm,
        ):
            UTT = {}
            for k in range(NB - 1):
                for j in range(k + 1, NB):
                    utr = pre.tile([128, 128], BF16, tag=f"ut{k}_{j}r",
                                   name=f"ut{k}_{j}r")
                    uti = pre.tile([128, 128], BF16, tag=f"ut{k}_{j}i",
                                   name=f"ut{k}_{j}i")
                    nc.sync.dma_start_transpose(
                        utr[:], scr["utdram"][128*k:128*(k+1),
                                              RW*j:RW*j+128])
                    nc.sync.dma_start_transpose(
                        uti[:], scr["utdram"][128*k:128*(k+1),
                                              RW*j+128:RW*j+256])
                    UTT[(k, j)] = (utr, uti)
            for k in range(NB - 1, -1, -1):
                if k < NB - 1:
                    P1a = pacc.tile([128, RW], F32, tag="bs_p1")
                    P2a = pacc.tile([128, RW], F32, tag="bs_p2")
                    nc.tensor.matmul(P1a[:], consts["nIb"][:], BF[k][:],
                                     start=True, stop=False)
                    for j in range(k + 1, NB):
                        utr, uti = UTT[(k, j)]
                        last = (j == NB - 1)
                        nc.tensor.matmul(P1a[:], utr[:], BF[j][:],
                                         start=False, stop=last)
                        nc.tensor.matmul(P2a[:], uti[:], BF[j][:],
                                         start=(j == k + 1), stop=last)
                    P2s = work.tile([128, RW], BF16, tag="bs_p2s")
                    nc.scalar.copy(P2s[:], P2a[:])
                    W = work.tile([128, RW], BF16, tag="bs_W")
                    nc.vector.tensor_tensor(out=W[:, 0:128],
                                            in0=P2s[:, 128:256],
                                            in1=P1a[:, 0:128],
                                            op=ALU.subtract)
                    nc.vector.scalar_tensor_tensor(
                        out=W[:, 128:256], in0=P1a[:, 128:256], scalar=-1.0,
                        in1=P2s[:, 0:128], op0=ALU.mult, op1=ALU.subtract)
                else:
                    W = BF[k]
                WB = work.tile([128, RW], BF16, tag="bs_WB")
                nc.vector.tensor_scalar(out=WB[:, 0:128], in0=W[:, 128:256],
                                        scalar1=-1.0, scalar2=None,
                                        op0=ALU.mult)
                nc.gpsimd.tensor_copy(WB[:, 128:256], W[:, 0:128])
                PS = pmm.tile([128, RW], F32, tag="bs_PS")
                nc.tensor.matmul(PS[:], ZR[k][:, 0:128], W[:],
                                 start=True, stop=False)
                nc.tensor.matmul(PS[:], ZR[k][:, 128:256], WB[:],
                                 start=False, stop=True)
                nc.scalar.copy(BF[k][:], PS[:])
                nc.sync.dma_start(xdbg[128*k:128*(k+1), :], BF[k][:])

    # ---------------- P4: total field, power model, weights ----------------
    late = ctx.enter_context(tc.tile_pool(name="late", bufs=1))
    dvec = late.tile([128, LB], F32)
    drep = late.tile([128, L16], F32)
    wrep_r = late.tile([128, L16], F32)
    wrep_i = late.tile([128, L16], F32)
    vsum = late.tile([128, 2 * NB], F32)
    lam = late.tile([128, 1], F32)
    st_ = late.tile([128, LB], F32)
    srep = late.tile([128, LPAD], F32)
    yrep = late.tile([128, LPAD], F32)
    with (
        tc.tile_pool(name="p4_work", bufs=2) as work,
        tc.tile_pool(name="p4_pacc", bufs=1, space="PSUM") as pacc,
        tc.tile_pool(name="p4_pmisc", bufs=1, space="PSUM") as pmisc,
    ):
        Ptf1 = pacc.tile([40, RW], F32, tag="tf_p1")
        Ptf2 = pacc.tile([40, RW], F32, tag="tf_p2")
        for i in range(NB):
            gt = work.tile([128, 80], F32, tag="tf_g")
            nc.sync.dma_start(gt[:], din["gscT"][128*i:128*(i+1), :])
            gtb = work.tile([128, 80], BF16, tag="tf_gb")
            mux.eng().tensor_copy(gtb[:], gt[:])
            stt = (i == 0); spp = (i == NB - 1)
            nc.tensor.matmul(Ptf1[:], gtb[:, 0:40], BF[i][:],
                             start=stt, stop=spp)
            nc.tensor.matmul(Ptf2[:], gtb[:, 40:80], BF[i][:],
                             start=stt, stop=spp)
        df = work.tile([40, 80], F32, tag="tf_df")
        nc.sync.dma_start(df[:], din["dfpack"][:])
        tfr = work.tile([40, 40], F32, tag="tfr")
        tfi = work.tile([40, 40], F32, tag="tfi")
        nc.vector.tensor_tensor(out=tfr[:], in0=df[:, 0:40],
                                in1=Ptf1[:, 0:40], op=ALU.add)
        nc.vector.tensor_tensor(out=tfr[:], in0=tfr[:],
                                in1=Ptf2[:, 128:168], op=ALU.subtract)
        nc.vector.tensor_tensor(out=tfi[:], in0=df[:, 40:80],
                                in1=Ptf1[:, 128:168], op=ALU.add)
        nc.vector.tensor_tensor(out=tfi[:], in0=tfi[:],
                                in1=Ptf2[:, 0:40], op=ALU.add)
        tfd = work.tile([40, 80], F32, tag="tf_out")
        nc.vector.tensor_copy(tfd[:, 0:40], tfr[:])
        nc.vector.tensor_copy(tfd[:, 40:80], tfi[:])
        nc.sync.dma_start(tfdbg[:], tfd[:])

        mask40 = work.tile([40, 40], F32, tag="mask40")
        zero40 = work.tile([40, 1], F32, tag="zero40")
        nc.vector.memset(mask40[:], 1.0)
        nc.vector.memset(zero40[:], 0.0)
        nc.vector.copy_predicated(mask40[:], idu_s[0:40, 0:40],
                                  zero40[:].broadcast_to([40, 40]))

        pw = work.tile([40, 40], F32, tag="pw")
        nc.vector.tensor_tensor(out=pw[:], in0=tfr[:], in1=tfr[:], op=ALU.mult)
        t1 = work.tile([40, 40], F32, tag="pw_t")
        nc.gpsimd.tensor_tensor(out=t1[:], in0=tfi[:], in1=tfi[:], op=ALU.mult)
        nc.vector.tensor_tensor(out=pw[:], in0=pw[:], in1=t1[:], op=ALU.add)
        amp = work.tile([40, 40], F32, tag="amp")
        nc.scalar.activation(amp[:], pw[:], AF.Sqrt)
        nc.vector.tensor_scalar(out=amp[:], in0=amp[:], scalar1=NOISE,
                                scalar2=None, op0=ALU.add)
        nc.scalar.activation(amp[:], amp[:], AF.Ln)
        tpi = work.tile([40, 40], F32, tag="tpi")
        nc.vector.tensor_scalar(out=tpi[:], in0=amp[:], scalar1=C20L,
                                scalar2=CADD, op0=ALU.mult, op1=ALU.add)
        rec = work.tile([40, 40], F32, tag="rec")
        nc.vector.reciprocal(rec[:], pw[:])
        wr = work.tile([40, 40], F32, tag="wr")
        nc.vector.scalar_tensor_tensor(out=wr[:], in0=tfr[:], scalar=SA,
                                       in1=rec[:], op0=ALU.mult, op1=ALU.mult)
        nc.vector.tensor_tensor(out=wr[:], in0=wr[:], in1=mask40[:],
                                op=ALU.mult)
        wi = work.tile([40, 40], F32, tag="wi")
        nc.vector.scalar_tensor_tensor(out=wi[:], in0=tfi[:], scalar=-SA,
                                       in1=rec[:], op0=ALU.mult, op1=ALU.mult)
        nc.vector.tensor_tensor(out=wi[:], in0=wi[:], in1=mask40[:],
                                op=ALU.mult)
        tp40 = work.tile([40, 40], F32, tag="tp40")
        nc.sync.dma_start(tp40[:], din["tp40"][:])
        d40 = work.tile([40, 40], F32, tag="d40")
        nc.vector.tensor_tensor(out=d40[:], in0=tp40[:], in1=tpi[:],
                                op=ALU.subtract)
        nc.vector.scalar_tensor_tensor(out=d40[:], in0=d40[:],
                                       scalar=1.0 / LOG10E20, in1=mask40[:],
                                       op0=ALU.mult, op1=ALU.mult)

        def t40_store(src, dram, name):
            pt = pmisc.tile([40, 40], F32, tag=f"t40p_{name}")
            nc.tensor.matmul(pt[:], src[:], id_s[0:40, 0:40], start=True,
                             stop=True)
            d = work.tile([40, 40], F32, tag=f"t40_{name}")
            nc.vector.tensor_copy(d[:], pt[:])
            nc.sync.dma_start(bass.AP(dram, 0, [[1, L16]]), d[:])

        t40_store(d40, scr["sdram"], "d")
        t40_store(wr, scr["wrdram"], "wr")
        t40_store(wi, scr["widram"], "wi")

        row = work.tile([1, L16], F32, tag="rowld")
        nc.sync.dma_start(row[:], bass.AP(scr["sdram"], 0, [[1, L16]]))
        nc.gpsimd.partition_broadcast(drep[:], row[:])
        row = work.tile([1, L16], F32, tag="rowld")
        nc.sync.dma_start(row[:], bass.AP(scr["wrdram"], 0, [[1, L16]]))
        nc.gpsimd.partition_broadcast(wrep_r[:], row[:])
        row = work.tile([1, L16], F32, tag="rowld")
        nc.sync.dma_start(row[:], bass.AP(scr["widram"], 0, [[1, L16]]))
        nc.gpsimd.partition_broadcast(wrep_i[:], row[:])

        nc.vector.memset(dvec[:], 0.0)
        nc.sync.dma_start(dvec[:, 0:12],
                          bass.AP(scr["sdram"], 0, [[1, 128], [128, 12]]))
        nc.sync.dma_start(dvec[0:64, 12:13],
                          bass.AP(scr["sdram"], 1536, [[1, 64]]))

    # ---------------- P5+P7: H build fused with Gram quads ----------------
    ga_pool = ctx.enter_context(tc.tile_pool(name="ga", bufs=1))
    GA = [ga_pool.tile([128, LPAD], F32, tag=f"ga{l}", name=f"ga{l}")
          for l in range(LB)]
    with (
        tc.tile_pool(name="p5_hq", bufs=1) as hqpool,
        tc.tile_pool(name="p5_work", bufs=1) as work,
        tc.tile_pool(name="p5_pg", bufs=4, space="PSUM") as pg,
    ):
        HQ = [hqpool.tile([128, LPAD], BF16, tag=f"hq{s}", name=f"hq{s}")
              for s in range(8)]
        for s in range(8):
            nc.gpsimd.memset(HQ[s][:, L16:LPAD], 0.0)
        nc.vector.memset(vsum[:], 0.0)

        def gram_round(q, nm):
            for l in range(LB):
                c0 = 128 * l
                for cc in range(c0, LPAD, 512):
                    cw = min(512, LPAD - cc)
                    pgt = pg.tile([128, 512], F32, tag="g_pg")
                    for m in range(nm):
                        nc.tensor.matmul(pgt[:, 0:cw], HQ[m][:, c0:c0+128],
                                         HQ[m][:, cc:cc+cw],
                                         start=(m == 0), stop=(m == nm - 1))
                    if q == 0:
                        nc.scalar.copy(GA[l][:, cc:cc+cw], pgt[:, 0:cw])
                    else:
                        nc.vector.tensor_tensor(out=GA[l][:, cc:cc+cw],
                                                in0=GA[l][:, cc:cc+cw],
                                                in1=pgt[:, 0:cw], op=ALU.add)

        for i in range(NB):
            sre = HQ[2 * (i % 4)]
            sim = HQ[2 * (i % 4) + 1]
            Gq = work.tile([128, 80], F32, tag="h_gq")
            Iq = work.tile([128, 80], F32, tag="h_iq")
            f_ap = fsc_s[:, i:i+1]
            nc.vector.tensor_scalar(out=Gq[:, 0:40], in0=BF[i][:, 168:208],
                                    scalar1=f_ap, scalar2=-1.0, op0=ALU.mult,
                                    op1=ALU.mult)
            nc.gpsimd.tensor_scalar(out=Gq[:, 40:80], in0=BF[i][:, 40:80],
                                    scalar1=f_ap, scalar2=None, op0=ALU.mult)
            nc.vector.tensor_scalar(out=Iq[:, 0:40], in0=BF[i][:, 128:168],
                                    scalar1=f_ap, scalar2=-1.0, op0=ALU.mult,
                                    op1=ALU.mult)
            nc.gpsimd.tensor_scalar(out=Iq[:, 40:80], in0=BF[i][:, 0:40],
                                    scalar1=f_ap, scalar2=None, op0=ALU.mult)
            GR3 = Gq[:, 0:40].rearrange("p (o r) -> p o r", o=1
                                        ).broadcast_to([128, 40, 40])
            GI3 = Gq[:, 40:80].rearrange("p (o r) -> p o r", o=1
                                         ).broadcast_to([128, 40, 40])
            IR3 = Iq[:, 0:40].rearrange("p (t o) -> p t o", o=1
                                        ).broadcast_to([128, 40, 40])
            II3 = Iq[:, 40:80].rearrange("p (t o) -> p t o", o=1
                                         ).broadcast_to([128, 40, 40])
            qr = work.tile([128, L16], F32, tag="h_qr")
            qi = work.tile([128, L16], F32, tag="h_qi")
            ta = work.tile([128, L16], F32, tag="h_ta")
            tb = work.tile([128, L16], F32, tag="h_tb")
            qr3 = qr[:].rearrange("p (t r) -> p t r", t=40)
            qi3 = qi[:].rearrange("p (t r) -> p t r", t=40)
            ta3 = ta[:].rearrange("p (t r) -> p t r", t=40)
            tb3 = tb[:].rearrange("p (t r) -> p t r", t=40)
            nc.vector.tensor_tensor(out=qr3, in0=GR3, in1=IR3, op=ALU.mult)
            nc.gpsimd.tensor_tensor(out=ta3, in0=GI3, in1=II3, op=ALU.mult)
            nc.gpsimd.tensor_tensor(out=qi3, in0=GI3, in1=IR3, op=ALU.mult)
            nc.vector.tensor_tensor(out=tb3, in0=GR3, in1=II3, op=ALU.mult)
            nc.vector.tensor_tensor(out=qr[:], in0=qr[:], in1=ta[:],
                                    op=ALU.subtract)
            nc.gpsimd.tensor_tensor(out=qi[:], in0=qi[:], in1=tb[:],
                                    op=ALU.add)
            # H rows: hr = qr*wr - qi*wi ; hi_stored = -(qr*wi + qi*wr)
            nc.vector.tensor_tensor(out=ta[:], in0=qr[:], in1=wrep_r[:],
                                    op=ALU.mult)
            nc.gpsimd.tensor_tensor(out=tb[:], in0=qi[:], in1=wrep_i[:],
                                    op=ALU.mult)
            nc.vector.tensor_tensor(out=sre[:, 0:L16], in0=ta[:], in1=tb[:],
                                    op=ALU.subtract)
            nc.gpsimd.tensor_tensor(out=ta[:], in0=qr[:], in1=wrep_i[:],
                                    op=ALU.mult)
            nc.vector.tensor_tensor(out=tb[:], in0=qi[:], in1=wrep_r[:],
                                    op=ALU.mult)
            nc.vector.scalar_tensor_tensor(out=sim[:, 0:L16], in0=ta[:],
                                           scalar=-1.0, in1=tb[:],
                                           op0=ALU.mult, op1=ALU.subtract)
            junk = work.tile([128, L16], BF16, tag="h_junk")
            nc.vector.scalar_tensor_tensor(
                out=junk[:], in0=sre[:, 0:L16], scalar=1.0, in1=drep[:],
                op0=ALU.mult, op1=ALU.mult, accum_out=vsum[:, i:i+1])
            junk2 = work.tile([128, L16], BF16, tag="h_junk2")
            nc.vector.scalar_tensor_tensor(
                out=junk2[:], in0=sim[:, 0:L16], scalar=1.0, in1=drep[:],
                op0=ALU.mult, op1=ALU.mult, accum_out=vsum[:, NB+i:NB+i+1])
            nc.sync.dma_start(scr["htdram"][128*i:128*(i+1), :], sre[:])
            nc.sync.dma_start(scr["htdram"][N+128*i:N+128*(i+1), :], sim[:])
            if i % 4 == 3:
                gram_round(i // 4, 8)
        gram_round(4, 4)

        vsq = work.tile([128, 2 * NB], F32, tag="vsq")
        nc.vector.tensor_tensor(out=vsq[:], in0=vsum[:], in1=vsum[:],
                                op=ALU.mult)
        vred = work.tile([128, 1], F32, tag="vred")
        nc.vector.tensor_reduce(vred[:], vsq[:], axis=AXX, op=ALU.add)
        nc.gpsimd.partition_all_reduce(vred[:], vred[:], 128,
                                       bass_isa.ReduceOp.add)
        nc.scalar.activation(lam[:], vred[:], AF.Sqrt)
        nc.vector.tensor_scalar(out=lam[:], in0=lam[:], scalar1=float(alpha),
                                scalar2=None, op0=ALU.mult)

    # ---------------- P8: scaled SPD block solve ----------------
    gr_pool = ctx.enter_context(tc.tile_pool(name="gr", bufs=1))
    GR = [gr_pool.tile([128, (LB - i) * 128], BF16, tag=f"gr{i}",
                       name=f"gr{i}") for i in range(LB)]
    BF2 = [gr_pool.tile([128, 1], BF16, tag=f"b2_{l}", name=f"b2_{l}")
           for l in range(LB)]
    ys = late.tile([128, LB], F32)
    with (
        tc.tile_pool(name="s_work", bufs=2) as work,
        tc.tile_pool(name="s_nw", bufs=1) as nwork,
        tc.tile_pool(name="s_pmm", bufs=1, space="PSUM") as pmm,
        tc.tile_pool(name="s_pup", bufs=1, space="PSUM") as pup,
        tc.tile_pool(name="s_pmisc", bufs=1, space="PSUM") as pmisc,
    ):
        # jacobi scaling vector from Gram diagonal
        gdiag = work.tile([128, LB], F32, tag="gdiag")
        for l in range(LB):
            t128 = work.tile([128, 128], F32, tag="gd_t")
            nc.vector.scalar_tensor_tensor(
                out=t128[:], in0=GA[l][:, 128*l:128*(l+1)], scalar=1.0,
                in1=id_s[:], op0=ALU.mult, op1=ALU.mult,
                accum_out=gdiag[:, l:l+1])
        nc.vector.tensor_scalar(out=gdiag[:], in0=gdiag[:], scalar1=lam[:],
                                scalar2=None, op0=ALU.add)
        nc.scalar.activation(st_[:], gdiag[:], AF.Sqrt)
        nc.vector.reciprocal(st_[:], st_[:])
        ps_ = pmisc.tile([LB, 128], F32, tag="s_ps")
        nc.tensor.matmul(ps_[:], st_[:], id_s[:], start=True, stop=True)
        s13 = work.tile([LB, 128], F32, tag="s13")
        nc.vector.tensor_copy(s13[:], ps_[:])
        nc.sync.dma_start(bass.AP(scr["srowdram"], 0, [[1, LPAD]]), s13[:])
        srow = work.tile([1, LPAD], F32, tag="srow")
        nc.sync.dma_start(srow[:], bass.AP(scr["srowdram"], 0, [[1, LPAD]]))
        nc.gpsimd.partition_broadcast(srep[:], srow[:])

        onesb = work.tile([128, 1], BF16, tag="onesb")
        nc.vector.memset(onesb[:], 1.0)
        for i in range(LB):
            for j in range(i, LB):
                nc.vector.scalar_tensor_tensor(
                    out=GR[i][:, (j-i)*128:(j-i)*128+128],
                    in0=GA[i][:, 128*j:128*(j+1)], scalar=st_[:, i:i+1],
                    in1=srep[:, 128*j:128*(j+1)], op0=ALU.mult, op1=ALU.mult)
            nc.vector.copy_predicated(GR[i][:, 0:128], idu_s[:],
                                      onesb[:].broadcast_to([128, 128]))
        dsc = work.tile([128, LB], F32, tag="dsc")
        nc.vector.tensor_tensor(out=dsc[:], in0=dvec[:], in1=st_[:],
                                op=ALU.mult)
        for l in range(LB):
            nc.gpsimd.tensor_copy(BF2[l][:], dsc[:, l:l+1])

        pending = []

        def drain(n):
            for _ in range(min(n, len(pending))):
                pending.pop(0)()

        for step in _newton_real_steps(nc, nwork, pmm, pmisc, GR[0][:, 0:128],
                                       consts, NEWTON_SPD):
            step()
        for k in range(LB):
            nr = LB - 1 - k
            if nr > 0:
                # LT row = V_k @ (pivot row right of diag), wide
                LTrow = work.tile([128, (LB - 1) * 128], BF16, tag="lt_row")
                Wk = nr * 128
                for c in range(0, Wk, 512):
                    w = min(512, Wk - c)
                    pl = pup.tile([128, 512], F32, tag="s_pl")
                    nc.tensor.matmul(pl[:, 0:w], GR[k][:, 0:128],
                                     GR[k][:, 128+c:128+c+w],
                                     start=True, stop=True)
                    nc.scalar.copy(LTrow[:, c:c+w], pl[:, 0:w])
            for i in range(k + 1, LB):
                lt = LTrow[:, (i-k-1)*128:(i-k)*128]
                pb = pmm.tile([128, 1], F32, tag="s_pb")
                nc.tensor.matmul(pb[:], lt, BF2[k][:], start=True, stop=True)
                nc.vector.tensor_tensor(out=BF2[i][:], in0=BF2[i][:],
                                        in1=pb[:], op=ALU.subtract)
                Wi = (LB - i) * 128
                for c in range(0, Wi, 512):
                    w = min(512, Wi - c)
                    pu = pup.tile([128, 512], F32, tag="s_pu")
                    nc.tensor.matmul(pu[:, 0:w], lt,
                                     GR[k][:, (i-k)*128+c:(i-k)*128+c+w],
                                     start=True, stop=True)
                    mux.i += 1
                    if mux.i & 1:
                        nc.vector.tensor_tensor(out=GR[i][:, c:c+w],
                                                in0=GR[i][:, c:c+w],
                                                in1=pu[:, 0:w],
                                                op=ALU.subtract)
                    else:
                        aptmp = work.tile([128, 512], BF16, tag="ap_tmp8")
                        nc.scalar.copy(aptmp[:, 0:w], pu[:, 0:w])
                        nc.gpsimd.tensor_tensor(out=GR[i][:, c:c+w],
                                                in0=GR[i][:, c:c+w],
                                                in1=aptmp[:, 0:w],
                                                op=ALU.subtract)
                if i == k + 1:
                    pending = list(_newton_real_steps(
                        nc, nwork, pmm, pmisc, GR[i][:, 0:128], consts,
                        NEWTON_SPD))
                    drain(3)
                else:
                    drain(3)
            drain(len(pending))

        # backward substitution
        for k in range(LB - 1, -1, -1):
            P1a = pmm.tile([128, 1], F32, tag="s_pb")
            nc.tensor.matmul(P1a[:], consts["nIb"][:], BF2[k][:],
                             start=True, stop=(k == LB - 1))
            for j in range(k + 1, LB):
                utt = work.tile([128, 128], BF16, tag="s_utt")
                nc.sync.dma_start_transpose(
                    utt[:], GR[k][:, (j-k)*128:(j-k+1)*128])
                nc.tensor.matmul(P1a[:], utt[:], BF2[j][:],
                                 start=False, stop=(j == LB - 1))
            W2 = work.tile([128, 1], BF16, tag="s_W2")
            nc.vector.tensor_copy(W2[:], P1a[:])
            PS = pmm.tile([128, 1], F32, tag="s_pb")
            nc.tensor.matmul(PS[:], GR[k][:, 0:128], W2[:],
                             start=True, stop=True)
            nc.vector.tensor_scalar(out=BF2[k][:], in0=PS[:], scalar1=-1.0,
                                    scalar2=None, op0=ALU.mult)
        for l in range(LB):
            nc.gpsimd.tensor_copy(ys[:, l:l+1], BF2[l][:])
        nc.vector.tensor_tensor(out=ys[:], in0=ys[:], in1=st_[:], op=ALU.mult)
        psy = pmisc.tile([LB, 128], F32, tag="y_ps")
        nc.tensor.matmul(psy[:], ys[:], id_s[:], start=True, stop=True)
        y13 = work.tile([LB, 128], F32, tag="y13")
        nc.vector.tensor_copy(y13[:], psy[:])
        nc.sync.dma_start(bass.AP(scr["yrowdram"], 0, [[1, LPAD]]), y13[:])
        yrow = work.tile([1, LPAD], F32, tag="yrow")
        nc.sync.dma_start(yrow[:], bass.AP(scr["yrowdram"], 0, [[1, LPAD]]))
        nc.gpsimd.partition_broadcast(yrep[:], yrow[:])

    # ---------------- P9: chi = Ht y ----------------
    with tc.tile_pool(name="p9_work", bufs=6) as work:
        chi = late.tile([128, 2 * NB], F32)
        for ch in range(2 * NB):
            htc = work.tile([128, LPAD], BF16, tag="c_htc")
            nc.sync.dma_start(htc[:], scr["htdram"][128*ch:128*(ch+1), :])
            junk = work.tile([128, LPAD], BF16, tag="c_junk")
            nc.vector.scalar_tensor_tensor(
                out=junk[:], in0=htc[:], scalar=1.0, in1=yrep[:],
                op0=ALU.mult, op1=ALU.mult, accum_out=chi[:, ch:ch+1])
        nc.sync.dma_start(bass.AP(out_chi, 0, [[1, 128], [128, 2 * NB]]),
                          chi[:])
    ctx.close()


_CACHED = {}


def kernel(epsilon_r_iter, chi_iter, total_power, alpha, grid_x, grid_y,
           direct_field, incident_field, G_freespace, G_freespace_scaled,
           sensor_links):
    eps = np.asarray(epsilon_r_iter)
    chi_it = np.asarray(chi_iter)
    tp = np.asarray(total_power, dtype=np.float32)
    alpha_f = float(np.asarray(alpha))
    gx = np.asarray(grid_x, dtype=np.float32)
    gy = np.asarray(grid_y, dtype=np.float32)
    df = np.asarray(direct_field)
    einc = np.asarray(incident_field)
    gfs = np.asarray(G_freespace)
    gsc = np.asarray(G_freespace_scaled)
    links = np.asarray(sensor_links)

    # this kernel assumes the canonical uniform link set (t-major, r != t)
    expect = np.array([[t, r] for t in range(TX) for r in range(RX) if r != t],
                      dtype=np.int32)
    assert links.shape == expect.shape and np.array_equal(links, expect), \
        "kernel specialized for the canonical sensor_links layout"

    x = gx.T.reshape(N).astype(np.float32)
    y = gy.T.reshape(N).astype(np.float32)
    scat = np.real(eps.T.reshape(N)).astype(np.float32)

    geomS = np.stack([np.ones(N, np.float32), -2.0*x, -2.0*y,
                      (x*x + y*y)]).astype(np.float32)
    geomR = np.stack([(x*x + y*y), x, y,
                      np.ones(N, np.float32)]).astype(np.float32)
    scat_t = scat.reshape(NB, 128).T.copy()

    bpack = np.zeros((N, RW), np.float32)
    bpack[:, 0:40] = -einc.real; bpack[:, 40:80] = -gfs.real
    bpack[:, 128:168] = -einc.imag; bpack[:, 168:208] = -gfs.imag
    gscT = np.concatenate([gsc.real.T, gsc.imag.T], axis=1).astype(np.float32)
    dfpack = np.concatenate([df.real, df.imag], axis=1).astype(np.float32)

    # total_power [RX-1, TX] -> [40, 40] with zeros on the diagonal
    tp40 = np.zeros((40, 40), np.float32)
    for t in range(TX):
        rs = [r for r in range(RX) if r != t]
        tp40[rs, t] = tp[:, t]

    key = alpha_f
    if key not in _CACHED:
        _CACHED[key] = build_program(alpha_f)
    nc = _CACHED[key]

    id128 = np.eye(128, dtype=np.float32)
    im = {
        "geomS": geomS, "geomR": geomR, "scat_t": scat_t, "bpack": bpack,
        "gscT": gscT, "dfpack": dfpack, "tp40": tp40,
        "id128": id128, "idu8": id128.astype(np.uint8),
    }
    import os as _os
    _tr = _os.environ.get("KTRACE", "0") == "1"
    res = run_bass_kernel_spmd(nc, [im] * 8, core_ids=list(range(8)),
                               trace=_tr)
    out = res.results[0]
    _CACHED["last"] = (res, out)

    chi = np.asarray(out["out_chi"], dtype=np.float32)
    dchi_r = chi[:N].reshape(M, M).T
    dchi_i = chi[N:].reshape(M, M).T
    chi_new = (chi_it + (dchi_r + 1j * dchi_i)).astype(np.complex64)
    return chi_new + 1.0, chi_new
